# revision 12
# baseline (speedup 1.0000x reference)
"""GQA prefill attention (B=2, S=2048, D=2048, H=32, KV=8, HD=64) on 8 trn2 cores.

Sharding: batch x head-group. Core c owns batch b=c%2 and head-group hg=c//2:
q-heads [8hg, 8hg+8), kv-heads {2hg, 2hg+1}; computes its partial of
out[b] = attn_out @ wo_hg; host sums the 4 partials per batch.

Device kernel (per core, bf16 matmuls / fp32 PSUM):
  QT[dh,s] per head-pair (two 64-row heads stacked on 128 partitions)
  KT[dh,s] per kv head, duplicated onto both partition halves (row tiling)
  V[k,dh] natural layout via PE transpose; 65th column of ones
  RoPE via pair-swap permutation matmul + elementwise cos/sin tables
  ST[k,q] = KT^T @ QT for a head pair: two concurrent row-tiled K=64 matmuls
  P = exp(ST/8) one ScalarE instr per (pair, kt) over both PSUM banks
  OT[dh|sum, q] += [V | 1]^T-chunks @ P  (N=512 streaming PV, no LDW churn)
  normalize: recip(denom row) -> PE ones-broadcast -> DVE multiply
  out_partial[s,:] = OTn-chunks^T @ wo_hg   (bf16 partial to HBM)
"""

import os
import sys

import numpy as np
import ml_dtypes

BF16 = ml_dtypes.bfloat16

B, S, D, H, KV, HD = 2, 2048, 2048, 32, 8, 64
NCORES = 8
HPC = 8            # q-heads per core
NPAIR = 4          # head pairs per core
QS_TILES = S // 512
KT_TILES = S // 128
DC = D // 128      # contraction chunks for the projections


def _host_prepare(x, wq, wk, wv, wo, freqs, mask):
    """Build per-core device inputs + the mask block schedule."""
    # RoPE tables in the [dh-on-partitions, s] layout used by QT/KT.
    # Two 64-row head copies stacked (head pairs live on 128 partitions).
    # rope: out[2j]   = t[2j] cos - t[2j+1] sin
    #       out[2j+1] = t[2j] sin + t[2j+1] cos
    # with swap(t)[d] = t[d^1]:  out[d] = t[d]*cos[d] + swap(t)[d]*sgn(d)*sin[d]
    c64 = np.cos(freqs.T).repeat(2, axis=0).astype(np.float64)  # [64, S]
    s64 = np.sin(freqs.T).repeat(2, axis=0).astype(np.float64)
    sgn = np.where(np.arange(HD) % 2 == 0, -1.0, 1.0)[:, None]
    cos_t = np.concatenate([c64, c64], axis=0).astype(BF16)           # [128, S]
    sin_t = np.concatenate([s64 * sgn, s64 * sgn], axis=0).astype(BF16)

    # Mask block schedule at [128 k x 512 q] granularity (same for all b, h).
    # Block (qs, kt): full (mask all zero), skip (all <= -30), or masked
    # (multiply exp'd P by exp(mask^T) tile). jlo = first visible 128-q
    # subblock (clean fully-masked prefix only).
    mt_tiles = []  # unique [128, 512] multiplier tiles
    mt_keys = {}
    sched = []  # per qs: list of (kt, mtile_idx | None, jlo)
    for qs in range(QS_TILES):
        lst = []
        for kt in range(KT_TILES):
            blk = mask[qs * 512:(qs + 1) * 512, kt * 128:(kt + 1) * 128]  # [q, k]
            if np.all(blk <= -30.0):
                continue
            jmasked = [np.all(blk[j * 128:(j + 1) * 128] <= -30.0) for j in range(4)]
            jlo = 0
            while jlo < 4 and jmasked[jlo]:
                jlo += 1
            if any(jmasked[jlo:]):
                jlo = 0
            vis = blk[jlo * 128:]
            if np.all(vis == 0.0):
                lst.append((kt, None, jlo))
                continue
            tile_np = np.exp(blk.T.astype(np.float64)).astype(BF16)  # [128k, 512q]
            key = tile_np.tobytes()
            if key not in mt_keys:
                mt_keys[key] = len(mt_tiles)
                mt_tiles.append(tile_np)
            lst.append((kt, mt_keys[key], jlo))
        # every 128-q subblock needs at least one full-width contributing kt
        # so its softmax denominator is well-defined; also the FIRST kt per
        # subblock must be full-width (start=True clears the whole OT bank)
        for j in range(4):
            if not any(e[2] <= j for e in lst):
                lst = [(kt, mi, 0) for (kt, mi, _) in lst]
                break
        if lst and lst[0][2] != 0:
            lst[0] = (lst[0][0], lst[0][1], 0)
        sched.append(lst)
    if not mt_tiles:  # keep the input well-formed even if no masked blocks
        mt_tiles.append(np.ones((128, 512), dtype=BF16))
    mt = np.stack(mt_tiles)  # [U, 128, 512]

    per_core = []
    for c in range(NCORES):
        b, hg = c % 2, c // 2
        per_core.append({
            "xT": np.ascontiguousarray(x[b].T).astype(BF16),            # [D, S]
            "wq": np.ascontiguousarray(
                wq[:, hg * 512:(hg + 1) * 512]).astype(BF16),           # [D, 512]
            "wkv": np.ascontiguousarray(np.concatenate(
                [wk[:, hg * 128:(hg + 1) * 128],
                 wv[:, hg * 128:(hg + 1) * 128]], axis=1)).astype(BF16),  # [D, 256]
            "wo": np.ascontiguousarray(
                wo[hg * 512:(hg + 1) * 512, :]).astype(BF16),           # [512, D]
            "cos": cos_t,
            "sin": sin_t,
            "mt": mt,
        })
    return per_core, sched, mt.shape[0]


def _build_program(sched, U):
    import concourse.bass as bass
    import concourse.mybir as mybir
    import concourse.tile as tile
    from concourse import bacc

    dt = mybir.dt
    bf, f32 = dt.bfloat16, dt.float32
    AF = mybir.ActivationFunctionType

    nc = bacc.Bacc("TRN2", target_bir_lowering=False, debug=False,
                   num_devices=NCORES)

    xT = nc.dram_tensor("xT", [D, S], bf, kind="ExternalInput")
    wq = nc.dram_tensor("wq", [D, 512], bf, kind="ExternalInput")
    wkv = nc.dram_tensor("wkv", [D, 256], bf, kind="ExternalInput")
    wo = nc.dram_tensor("wo", [512, D], bf, kind="ExternalInput")
    cos = nc.dram_tensor("cos", [128, S], bf, kind="ExternalInput")
    sin = nc.dram_tensor("sin", [128, S], bf, kind="ExternalInput")
    mt = nc.dram_tensor("mt", [U, 128, 512], bf, kind="ExternalInput")
    out = nc.dram_tensor("out", [S, D], bf, kind="ExternalOutput")

    # pair-swap permutation (block-diag over the two stacked 64-row heads)
    perm_np = np.zeros((128, 128), dtype=BF16)
    for d in range(128):
        perm_np[d ^ 1, d] = 1
    perm_dram = nc.inline_tensor(np.ascontiguousarray(perm_np), name="perm")
    ident_dram = nc.inline_tensor(np.eye(128, dtype=BF16), name="ident")

    with tile.TileContext(nc) as tc:
        with tc.tile_pool(name="const", bufs=1) as cp:
            wq_sb = cp.tile([128, DC, 512], bf)
            nc.sync.dma_start(wq_sb[:], wq.ap().rearrange("(c p) m -> p c m", p=128))
            wkv_sb = cp.tile([128, DC, 256], bf)
            nc.sync.dma_start(wkv_sb[:], wkv.ap().rearrange("(c p) m -> p c m", p=128))
            wo_sb = cp.tile([128, NPAIR, D], bf)
            nc.sync.dma_start(wo_sb[:], wo.ap().rearrange("(g p) n -> p g n", p=128))
            cos_sb = cp.tile([128, S], bf)
            nc.sync.dma_start(cos_sb[:], cos.ap())
            sin_sb = cp.tile([128, S], bf)
            nc.sync.dma_start(sin_sb[:], sin.ap())
            mt_sb = cp.tile([128, U, 512], bf)
            nc.sync.dma_start(mt_sb[:], mt.ap().rearrange("u p q -> p u q"))
            perm_sb = cp.tile([128, 128], bf)
            nc.sync.dma_start(perm_sb[:], perm_dram.ap())
            ident_sb = cp.tile([128, 128], bf)
            nc.sync.dma_start(ident_sb[:], ident_dram.ap())
            ones_sb = cp.tile([128, 64], f32)
            nc.vector.memset(ones_sb[:], 1.0)

            qt_sb = cp.tile([128, NPAIR, S], bf)    # [dh-pair, pair, s]
            kt_sb = cp.tile([128, 2, S], bf)        # [dh dup'd halves, kv, s]
            vone_sb = cp.tile([128, 2, KT_TILES, HD + 1], bf)  # [k%128, kv, kt, dh|1]
            nc.vector.memset(vone_sb[:, :, :, HD:HD + 1], 1.0)

            # ---------------- phase 1: projections + rope ----------------
            with (
                tc.tile_pool(name="xt", bufs=2) as xp,
                tc.tile_pool(name="raw", bufs=2) as rawp,
                tc.tile_pool(name="rtmp", bufs=2) as rtp,
                tc.tile_pool(name="ps_pr", bufs=1, space="PSUM") as ppr,
                tc.tile_pool(name="ps_sw", bufs=1, space="PSUM") as psw,
                tc.tile_pool(name="ps_vt", bufs=1, space="PSUM") as pvt,
            ):
                for st in range(S // 512):
                    s0 = st * 512
                    xbig = xp.tile([128, DC, 512], bf)
                    nc.sync.dma_start(
                        xbig[:],
                        xT.ap().rearrange("(c p) s -> p c s", p=128)[:, :, s0:s0 + 512],
                    )
                    qps = [ppr.tile([128, 512], f32, tag=f"q{i}", name=f"qp{i}")
                           for i in range(NPAIR)]
                    kp = ppr.tile([128, 512], f32, tag="kp")
                    vp = ppr.tile([128, 512], f32, tag="vp")
                    for dc in range(DC):
                        st_, sp_ = (dc == 0), (dc == DC - 1)
                        for i in range(NPAIR):
                            nc.tensor.matmul(qps[i][:],
                                             lhsT=wq_sb[:, dc, i * 128:(i + 1) * 128],
                                             rhs=xbig[:, dc, :], start=st_, stop=sp_)
                        nc.tensor.matmul(kp[:], lhsT=wkv_sb[:, dc, 0:128],
                                         rhs=xbig[:, dc, :], start=st_, stop=sp_)
                        nc.tensor.matmul(vp[:], lhsT=wkv_sb[:, dc, 128:256],
                                         rhs=xbig[:, dc, :], start=st_, stop=sp_)
                    # raw copies to SBUF (also the swap-matmul inputs)
                    qrs = []
                    for i in range(NPAIR):
                        qr = rawp.tile([128, 512], bf, tag=f"q{i}r", name=f"q{i}r")
                        nc.vector.tensor_copy(qr[:], qps[i][:])
                        qrs.append(qr)
                    kr = rawp.tile([128, 512], bf, tag="kr")
                    nc.scalar.copy(kr[:], kp[:])
                    vr = rawp.tile([128, 512], bf, tag="vr")
                    nc.scalar.copy(vr[:], vp[:])
                    # V: transpose VT rows back to natural [k, dh]
                    for kv in range(2):
                        pl = kv * 64
                        for j in range(4):
                            vtp = pvt.tile([128, HD], bf)
                            nc.tensor.transpose(
                                vtp[:], vr[pl:pl + 64, j * 128:(j + 1) * 128],
                                ident_sb[pl:pl + 64, pl:pl + 64])
                            nc.vector.tensor_copy(vone_sb[:, kv, 4 * st + j, 0:HD],
                                                  vtp[:])
                    # rope Q (pairs stacked on 128 partitions)
                    for i, qr in enumerate(qrs):
                        swp = psw.tile([128, 512], f32, tag="sw")
                        nc.tensor.matmul(swp[:], lhsT=perm_sb[:], rhs=qr[:],
                                         start=True, stop=True)
                        t_sin = rtp.tile([128, 512], bf, tag="tsin")
                        nc.vector.tensor_mul(t_sin[:], swp[:], sin_sb[:, s0:s0 + 512])
                        t_cos = rtp.tile([128, 512], bf, tag="tcos")
                        nc.vector.tensor_mul(t_cos[:], qr[:], cos_sb[:, s0:s0 + 512])
                        nc.vector.tensor_add(qt_sb[:, i, s0:s0 + 512],
                                             t_sin[:], t_cos[:])
                    # rope K (both kv heads stacked); write into the dup'd
                    # halves, then DMA-duplicate across partition halves
                    ksw = psw.tile([128, 512], f32, tag="sw")
                    nc.tensor.matmul(ksw[:], lhsT=perm_sb[:], rhs=kr[:],
                                     start=True, stop=True)
                    k_sin = rtp.tile([128, 512], bf, tag="tsin")
                    nc.vector.tensor_mul(k_sin[:], ksw[:], sin_sb[:, s0:s0 + 512])
                    k_cos = rtp.tile([128, 512], bf, tag="tcos")
                    nc.vector.tensor_mul(k_cos[:], kr[:], cos_sb[:, s0:s0 + 512])
                    nc.vector.tensor_add(kt_sb[0:64, 0, s0:s0 + 512],
                                         k_sin[0:64, :], k_cos[0:64, :])
                    nc.vector.tensor_add(kt_sb[64:128, 1, s0:s0 + 512],
                                         k_sin[64:128, :], k_cos[64:128, :])
                    nc.sync.dma_start(kt_sb[64:128, 0, s0:s0 + 512],
                                      kt_sb[0:64, 0, s0:s0 + 512])
                    nc.sync.dma_start(kt_sb[0:64, 1, s0:s0 + 512],
                                      kt_sb[64:128, 1, s0:s0 + 512])

            # ---------------- phase 2: attention + wo ----------------
            with (
                tc.tile_pool(name="pp", bufs=3) as ppool,
                tc.tile_pool(name="rcp", bufs=2) as rcp,
                tc.tile_pool(name="otn", bufs=2) as otp,
                tc.tile_pool(name="wsb", bufs=3) as wsp,
                tc.tile_pool(name="ps_s", bufs=2, space="PSUM") as pss,
                tc.tile_pool(name="ps_o", bufs=1, space="PSUM") as pso,
                tc.tile_pool(name="ps_r", bufs=1, space="PSUM") as psr,
                tc.tile_pool(name="ps_w", bufs=1, space="PSUM") as psw2,
            ):
                for qs in range(QS_TILES):
                    q0 = qs * 512
                    kts = sched[qs]
                    last_kt = max(e[0] for e in kts)
                    otn_t = otp.tile([128, NPAIR, 512], bf)  # [dh-pair, pair, q]
                    for p in range(NPAIR):
                        kv = p // 2
                        ot0 = pso.tile([HD + 1, 512], f32, tag="ot0", name="ot0")
                        ot1 = pso.tile([HD + 1, 512], f32, tag="ot1", name="ot1")
                        for kt, mi, jlo in kts:
                            nq = 512 - jlo * 128
                            ql = q0 + jlo * 128
                            spp = pss.tile([128, 2, 512], f32)
                            for h2 in range(2):
                                pl = h2 * 64
                                nc.tensor.matmul(
                                    spp[:, h2, jlo * 128:512],
                                    lhsT=kt_sb[pl:pl + 64, kv,
                                               kt * 128:(kt + 1) * 128],
                                    rhs=qt_sb[pl:pl + 64, p, ql:q0 + 512],
                                    start=True, stop=True)
                            pt = ppool.tile([128, 2, 512], bf)
                            nc.scalar.activation(pt[:, :, jlo * 128:512],
                                                 spp[:, :, jlo * 128:512], AF.Exp,
                                                 scale=1.0 / np.sqrt(HD))
                            if mi is not None:
                                for h2 in range(2):
                                    nc.vector.tensor_mul(
                                        pt[:, h2, jlo * 128:512],
                                        pt[:, h2, jlo * 128:512],
                                        mt_sb[:, mi, jlo * 128:512])
                            for h2, ot in ((0, ot0), (1, ot1)):
                                nc.tensor.matmul(
                                    ot[:, jlo * 128:512],
                                    lhsT=vone_sb[:, kv, kt, :],
                                    rhs=pt[:, h2, jlo * 128:512],
                                    start=(kt == kts[0][0]),
                                    stop=(kt == last_kt))
                        # normalize: denom row 64 -> recip -> PE ones-broadcast
                        # to 64 partitions -> copy to SBUF -> DVE multiply.
                        # Odd heads land on partitions 64:128 of otn_t via an
                        # SBUF->SBUF DMA (engines cannot cross partitions).
                        for h2, ot in ((0, ot0), (1, ot1)):
                            rc = rcp.tile([128, 512], f32, tag="rc")
                            nc.vector.reciprocal(rc[64:65, :], ot[64:65, :])
                            rb = psr.tile([64, 512], f32, tag="rb")
                            nc.tensor.matmul(rb[:], lhsT=ones_sb[64:65, :],
                                             rhs=rc[64:65, :],
                                             start=True, stop=True)
                            rbs = rcp.tile([64, 512], f32, tag="rbs")
                            nc.scalar.copy(rbs[:], rb[:])
                            if h2 == 0:
                                nc.vector.tensor_mul(otn_t[0:64, p, :],
                                                     ot[0:64, :], rbs[:])
                            else:
                                otmp = rcp.tile([64, 512], bf, tag="otmp")
                                nc.vector.tensor_mul(otmp[:], ot[0:64, :], rbs[:])
                                nc.sync.dma_start(otn_t[64:128, p, :], otmp[:])
                    # wo for this q-stripe
                    for j in range(4):
                        sq0 = q0 + j * 128
                        for nb in range(4):
                            wp = psw2.tile([128, 512], f32)
                            for p in range(NPAIR):
                                nc.tensor.matmul(
                                    wp[:],
                                    lhsT=otn_t[:, p, j * 128:(j + 1) * 128],
                                    rhs=wo_sb[:, p, nb * 512:(nb + 1) * 512],
                                    start=(p == 0), stop=(p == NPAIR - 1))
                            wsb = wsp.tile([128, 512], bf)
                            if (j + nb) % 2 == 0:
                                nc.vector.tensor_copy(wsb[:], wp[:])
                            else:
                                nc.scalar.copy(wsb[:], wp[:])
                            nc.sync.dma_start(
                                out.ap()[sq0:sq0 + 128, nb * 512:(nb + 1) * 512],
                                wsb[:])
    nc.compile()
    return nc


def kernel(x, wq, wk, wv, wo, freqs, mask, start_pos):
    sys.path.insert(0, "/opt/trn_rl_repo")
    from concourse.bass_utils import run_bass_kernel_spmd

    x = np.asarray(x, dtype=np.float32)
    per_core, sched, U = _host_prepare(
        x, np.asarray(wq, np.float32), np.asarray(wk, np.float32),
        np.asarray(wv, np.float32), np.asarray(wo, np.float32),
        np.asarray(freqs, np.float32), np.asarray(mask, np.float32))

    nc = _build_program(sched, U)

    trace = bool(int(os.environ.get("BASSKERNEL_TRACE", "0")))
    if trace and "antenv.axon_hooks" not in sys.modules:
        # profile-hook shim (the trimmed antenv package lacks axon_hooks)
        try:
            import types

            if "/root/.axon_site" not in sys.path:
                sys.path.insert(0, "/root/.axon_site")
            from trn_agent_boot.trn_boot import _ntff_profile_via_ctypes

            _hook = _ntff_profile_via_ctypes("/opt/axon/libaxon_pjrt.so")
            _mod = types.ModuleType("antenv.axon_hooks")
            _mod.get_axon_ntff_profile_hook = lambda: _hook
            _mod.set_axon_ntff_profile_hook = lambda h: None
            sys.modules["antenv.axon_hooks"] = _mod
        except Exception:
            trace = False
    res = run_bass_kernel_spmd(nc, per_core, core_ids=list(range(NCORES)),
                               trace=trace)
    if trace:
        kernel._last_exec_time_ns = res.exec_time_ns
        kernel._last_profile = res.profile_json
    acc = np.zeros((B, S, D), np.float64)
    for c in range(NCORES):
        acc[c % 2] += res.results[c]["out"].astype(np.float64)
    return acc.astype(np.float32)


# revision 22
# speedup vs baseline: 1.2308x; 1.2308x over previous
"""GQA prefill attention (B=2, S=2048, D=2048, H=32, KV=8, HD=64) on 8 trn2 cores.

Sharding: batch x head-group. Core c owns batch b=c%2 and head-group hg=c//2:
q-heads [8hg, 8hg+8), kv-heads {2hg, 2hg+1}; computes its partial of
out[b] = attn_out @ wo_hg; host sums the 4 partials per batch.

Device kernel (per core, bf16 matmuls / fp32 PSUM):
  QT[dh,s] per head-pair (two 64-row heads stacked on 128 partitions)
  KT[dh,s] per kv head, duplicated onto both partition halves (row tiling)
  V[k,dh] natural layout via PE transpose; 65th column of ones
  RoPE via pair-swap permutation matmul + elementwise cos/sin tables
  ST[k,q] = KT^T @ QT for a head pair: two concurrent row-tiled K=64 matmuls
  P = exp(ST/8) one ScalarE instr per (pair, kt) over both PSUM banks
  OT[dh|sum, q] += [V | 1]^T-chunks @ P  (N=512 streaming PV, no LDW churn)
  normalize: recip(denom row) -> PE ones-broadcast -> DVE multiply
  out_partial[s,:] = OTn-chunks^T @ wo_hg   (bf16 partial to HBM)
"""

import os
import sys

import numpy as np
import ml_dtypes

BF16 = ml_dtypes.bfloat16

B, S, D, H, KV, HD = 2, 2048, 2048, 32, 8, 64
NCORES = 8
HPC = 8            # q-heads per core
NPAIR = 4          # head pairs per core
QS_TILES = S // 512
KT_TILES = S // 128
DC = D // 128      # contraction chunks for the projections

# per-pair wo row order: odd head first (otn_t partitions 0:64), even second
_WO_PERM = np.concatenate(
    [np.concatenate([np.arange((2 * p + 1) * 64, (2 * p + 2) * 64),
                     np.arange(2 * p * 64, (2 * p + 1) * 64)])
     for p in range(4)])


def _host_prepare(x, wq, wk, wv, wo, freqs, mask):
    """Build per-core device inputs + the mask block schedule."""
    # RoPE tables in the [dh-on-partitions, s] layout used by QT/KT.
    # Two 64-row head copies stacked (head pairs live on 128 partitions).
    # rope: out[2j]   = t[2j] cos - t[2j+1] sin
    #       out[2j+1] = t[2j] sin + t[2j+1] cos
    # with swap(t)[d] = t[d^1]:  out[d] = t[d]*cos[d] + swap(t)[d]*sgn(d)*sin[d]
    c64 = np.cos(freqs.T).repeat(2, axis=0).astype(np.float64)  # [64, S]
    s64 = np.sin(freqs.T).repeat(2, axis=0).astype(np.float64)
    sgn = np.where(np.arange(HD) % 2 == 0, -1.0, 1.0)[:, None]
    cos_t = np.concatenate([c64, c64], axis=0).astype(BF16)           # [128, S]
    sin_t = np.concatenate([s64 * sgn, s64 * sgn], axis=0).astype(BF16)

    # Mask block schedule at [128 k x 512 q] granularity (same for all b, h).
    # Block (qs, kt): full (mask all zero), skip (all <= -30), or masked
    # (multiply exp'd P by exp(mask^T) tile). jlo = first visible 128-q
    # subblock (clean fully-masked prefix only).
    mt_tiles = []  # unique [128, 512] multiplier tiles
    mt_keys = {}
    sched = []  # per qs: list of (kt, mtile_idx | None, jlo)
    for qs in range(QS_TILES):
        lst = []
        for kt in range(KT_TILES):
            blk = mask[qs * 512:(qs + 1) * 512, kt * 128:(kt + 1) * 128]  # [q, k]
            if np.all(blk <= -30.0):
                continue
            jmasked = [np.all(blk[j * 128:(j + 1) * 128] <= -30.0) for j in range(4)]
            jlo = 0
            while jlo < 4 and jmasked[jlo]:
                jlo += 1
            if any(jmasked[jlo:]):
                jlo = 0
            vis = blk[jlo * 128:]
            if np.all(vis == 0.0):
                lst.append((kt, None, jlo))
                continue
            tile_np = np.exp(blk.T.astype(np.float64)).astype(BF16)  # [128k, 512q]
            key = tile_np.tobytes()
            if key not in mt_keys:
                mt_keys[key] = len(mt_tiles)
                mt_tiles.append(tile_np)
            lst.append((kt, mt_keys[key], jlo))
        # every 128-q subblock needs at least one full-width contributing kt
        # so its softmax denominator is well-defined; also the FIRST kt per
        # subblock must be full-width (start=True clears the whole OT bank)
        for j in range(4):
            if not any(e[2] <= j for e in lst):
                lst = [(kt, mi, 0) for (kt, mi, _) in lst]
                break
        if lst and lst[0][2] != 0:
            lst[0] = (lst[0][0], lst[0][1], 0)
        sched.append(lst)
    if not mt_tiles:  # keep the input well-formed even if no masked blocks
        mt_tiles.append(np.ones((128, 512), dtype=BF16))
    mt = np.stack(mt_tiles)  # [U, 128, 512]

    per_core = []
    for c in range(NCORES):
        b, hg = c % 2, c // 2
        per_core.append({
            "xT": np.ascontiguousarray(x[b].T).astype(BF16),            # [D, S]
            "wq": np.ascontiguousarray(
                wq[:, hg * 512:(hg + 1) * 512]).astype(BF16),           # [D, 512]
            "wkv": np.ascontiguousarray(np.concatenate(
                [wk[:, hg * 128:(hg + 1) * 128],
                 wv[:, hg * 128:(hg + 1) * 128]], axis=1)).astype(BF16),  # [D, 256]
            # wo rows permuted to match otn_t layout: pair p holds head 2p+1
            # on partitions 0:64 and head 2p on partitions 64:128
            "wo": np.ascontiguousarray(
                wo[hg * 512:(hg + 1) * 512, :][_WO_PERM, :]).astype(BF16),  # [512, D]
            "cos": cos_t,
            "sin": sin_t,
            "mt": mt,
        })
    return per_core, sched, mt.shape[0]


def _build_program(sched, U):
    import concourse.bass as bass
    import concourse.mybir as mybir
    import concourse.tile as tile
    from concourse import bacc

    dt = mybir.dt
    bf, f32 = dt.bfloat16, dt.float32
    AF = mybir.ActivationFunctionType

    nc = bacc.Bacc("TRN2", target_bir_lowering=False, debug=False,
                   num_devices=NCORES)

    xT = nc.dram_tensor("xT", [D, S], bf, kind="ExternalInput")
    wq = nc.dram_tensor("wq", [D, 512], bf, kind="ExternalInput")
    wkv = nc.dram_tensor("wkv", [D, 256], bf, kind="ExternalInput")
    wo = nc.dram_tensor("wo", [512, D], bf, kind="ExternalInput")
    cos = nc.dram_tensor("cos", [128, S], bf, kind="ExternalInput")
    sin = nc.dram_tensor("sin", [128, S], bf, kind="ExternalInput")
    mt = nc.dram_tensor("mt", [U, 128, 512], bf, kind="ExternalInput")
    out = nc.dram_tensor("out", [S, D], bf, kind="ExternalOutput")

    # pair-swap permutation (block-diag over the two stacked 64-row heads)
    perm_np = np.zeros((128, 128), dtype=BF16)
    for d in range(128):
        perm_np[d ^ 1, d] = 1
    perm_dram = nc.inline_tensor(np.ascontiguousarray(perm_np), name="perm")
    ident_dram = nc.inline_tensor(np.eye(128, dtype=BF16), name="ident")

    with tile.TileContext(nc) as tc:
        with tc.tile_pool(name="const", bufs=1) as cp:
            # weight loads split per contraction chunk so the first projection
            # matmuls can start as soon as chunk 0 lands
            wq_sb = cp.tile([128, DC, 512], bf)
            wkv_sb = cp.tile([128, DC, 256], bf)
            wq_r = wq.ap().rearrange("(c p) m -> p c m", p=128)
            wkv_r = wkv.ap().rearrange("(c p) m -> p c m", p=128)
            for dc in range(DC):
                nc.sync.dma_start(wq_sb[:, dc, :], wq_r[:, dc, :])
                nc.sync.dma_start(wkv_sb[:, dc, :], wkv_r[:, dc, :])
            perm_sb = cp.tile([128, 128], bf)
            nc.sync.dma_start(perm_sb[:], perm_dram.ap())
            ident_sb = cp.tile([128, 128], bf)
            nc.sync.dma_start(ident_sb[:], ident_dram.ap())
            cos_sb = cp.tile([128, S], bf)
            nc.sync.dma_start(cos_sb[:], cos.ap())
            sin_sb = cp.tile([128, S], bf)
            nc.sync.dma_start(sin_sb[:], sin.ap())
            mt_sb = cp.tile([128, U, 512], bf)
            nc.sync.dma_start(mt_sb[:], mt.ap().rearrange("u p q -> p u q"))
            wo_sb = cp.tile([128, NPAIR, D], bf)
            nc.sync.dma_start(wo_sb[:], wo.ap().rearrange("(g p) n -> p g n", p=128))
            qt_sb = cp.tile([128, NPAIR, S], bf)    # [dh-pair, pair, s]
            kt_sb = cp.tile([128, 2, S], bf)        # [dh dup'd halves, kv, s]
            # PV stationary operand per (kv, kt): [ones(64) | V(64)] so the
            # softmax denominator lands on PSUM partitions 0:64 (partition 0
            # feeds the gpsimd broadcast) and O^T on partitions 64:128
            vone_sb = cp.tile([128, 2, KT_TILES, 128], bf)  # [k%128, kv, kt, 1|dh]
            nc.vector.memset(vone_sb[:, :, :, 0:HD], 1.0)

            # ---------------- phase 1: projections + rope ----------------
            with (
                tc.tile_pool(name="xt", bufs=2) as xp,
                tc.tile_pool(name="raw", bufs=2) as rawp,
                tc.tile_pool(name="rtmp", bufs=2) as rtp,
                tc.tile_pool(name="ps_pr", bufs=1, space="PSUM") as ppr,
                tc.tile_pool(name="ps_sw", bufs=1, space="PSUM") as psw,
                tc.tile_pool(name="ps_vt", bufs=1, space="PSUM") as pvt,
            ):
                for st in range(S // 512):
                    s0 = st * 512
                    xbig = xp.tile([128, DC, 512], bf)
                    nc.sync.dma_start(
                        xbig[:],
                        xT.ap().rearrange("(c p) s -> p c s", p=128)[:, :, s0:s0 + 512],
                    )
                    qps = [ppr.tile([128, 512], f32, tag=f"q{i}", name=f"qp{i}")
                           for i in range(NPAIR)]
                    kp = ppr.tile([128, 512], f32, tag="kp")
                    vp = ppr.tile([128, 512], f32, tag="vp")
                    for dc in range(DC):
                        st_, sp_ = (dc == 0), (dc == DC - 1)
                        for i in range(NPAIR):
                            nc.tensor.matmul(qps[i][:],
                                             lhsT=wq_sb[:, dc, i * 128:(i + 1) * 128],
                                             rhs=xbig[:, dc, :], start=st_, stop=sp_)
                        nc.tensor.matmul(kp[:], lhsT=wkv_sb[:, dc, 0:128],
                                         rhs=xbig[:, dc, :], start=st_, stop=sp_)
                        nc.tensor.matmul(vp[:], lhsT=wkv_sb[:, dc, 128:256],
                                         rhs=xbig[:, dc, :], start=st_, stop=sp_)
                    # raw copies to SBUF (also the swap-matmul inputs)
                    qrs = []
                    for i in range(NPAIR):
                        qr = rawp.tile([128, 512], bf, tag=f"q{i}r", name=f"q{i}r")
                        nc.vector.tensor_copy(qr[:], qps[i][:])
                        qrs.append(qr)
                    kr = rawp.tile([128, 512], bf, tag="kr")
                    nc.scalar.copy(kr[:], kp[:])
                    vr = rawp.tile([128, 512], bf, tag="vr")
                    nc.scalar.copy(vr[:], vp[:])
                    # V: transpose VT rows back to natural [k, dh]
                    for kv in range(2):
                        pl = kv * 64
                        for j in range(4):
                            vtp = pvt.tile([128, HD], bf)
                            nc.tensor.transpose(
                                vtp[:], vr[pl:pl + 64, j * 128:(j + 1) * 128],
                                ident_sb[pl:pl + 64, pl:pl + 64])
                            nc.vector.tensor_copy(
                                vone_sb[:, kv, 4 * st + j, HD:128], vtp[:])
                    # rope Q (pairs stacked on 128 partitions)
                    for i, qr in enumerate(qrs):
                        swp = psw.tile([128, 512], f32, tag="sw")
                        nc.tensor.matmul(swp[:], lhsT=perm_sb[:], rhs=qr[:],
                                         start=True, stop=True)
                        t_sin = rtp.tile([128, 512], bf, tag="tsin")
                        nc.vector.tensor_mul(t_sin[:], swp[:], sin_sb[:, s0:s0 + 512])
                        t_cos = rtp.tile([128, 512], bf, tag="tcos")
                        nc.vector.tensor_mul(t_cos[:], qr[:], cos_sb[:, s0:s0 + 512])
                        nc.vector.tensor_add(qt_sb[:, i, s0:s0 + 512],
                                             t_sin[:], t_cos[:])
                    # rope K (both kv heads stacked); write into the dup'd
                    # halves, then DMA-duplicate across partition halves
                    ksw = psw.tile([128, 512], f32, tag="sw")
                    nc.tensor.matmul(ksw[:], lhsT=perm_sb[:], rhs=kr[:],
                                     start=True, stop=True)
                    k_sin = rtp.tile([128, 512], bf, tag="tsin")
                    nc.vector.tensor_mul(k_sin[:], ksw[:], sin_sb[:, s0:s0 + 512])
                    k_cos = rtp.tile([128, 512], bf, tag="tcos")
                    nc.vector.tensor_mul(k_cos[:], kr[:], cos_sb[:, s0:s0 + 512])
                    nc.vector.tensor_add(kt_sb[0:64, 0, s0:s0 + 512],
                                         k_sin[0:64, :], k_cos[0:64, :])
                    nc.vector.tensor_add(kt_sb[64:128, 1, s0:s0 + 512],
                                         k_sin[64:128, :], k_cos[64:128, :])
                    nc.sync.dma_start(kt_sb[64:128, 0, s0:s0 + 512],
                                      kt_sb[0:64, 0, s0:s0 + 512])
                    nc.sync.dma_start(kt_sb[0:64, 1, s0:s0 + 512],
                                      kt_sb[64:128, 1, s0:s0 + 512])

            # ---------------- phase 2: attention + wo ----------------
            with (
                tc.tile_pool(name="pp", bufs=3) as ppool,
                tc.tile_pool(name="rcp", bufs=2) as rcp,
                tc.tile_pool(name="otn", bufs=2) as otp,
                tc.tile_pool(name="wsb", bufs=3) as wsp,
                tc.tile_pool(name="ps_s", bufs=2, space="PSUM") as pss,
                tc.tile_pool(name="ps_o", bufs=1, space="PSUM") as pso,
                tc.tile_pool(name="ps_w", bufs=2, space="PSUM") as psw2,
            ):
                for qs in range(QS_TILES):
                    q0 = qs * 512
                    kts = sched[qs]
                    last_kt = max(e[0] for e in kts)
                    otn_t = otp.tile([128, NPAIR, 512], bf)  # [dh-pair, pair, q]
                    for p in range(NPAIR):
                        kv = p // 2
                        ot0 = pso.tile([128, 512], f32, tag="ot0", name="ot0")
                        ot1 = pso.tile([128, 512], f32, tag="ot1", name="ot1")
                        for kt, mi, jlo in kts:
                            nq = 512 - jlo * 128
                            ql = q0 + jlo * 128
                            spp = pss.tile([128, 2, 512], f32)
                            for h2 in range(2):
                                pl = h2 * 64
                                nc.tensor.matmul(
                                    spp[:, h2, jlo * 128:512],
                                    lhsT=kt_sb[pl:pl + 64, kv,
                                               kt * 128:(kt + 1) * 128],
                                    rhs=qt_sb[pl:pl + 64, p, ql:q0 + 512],
                                    start=True, stop=True)
                            pt = ppool.tile([128, 2, 512], bf)
                            nc.scalar.activation(pt[:, :, jlo * 128:512],
                                                 spp[:, :, jlo * 128:512], AF.Exp,
                                                 scale=1.0 / np.sqrt(HD))
                            if mi is not None:
                                for h2 in range(2):
                                    nc.vector.tensor_mul(
                                        pt[:, h2, jlo * 128:512],
                                        pt[:, h2, jlo * 128:512],
                                        mt_sb[:, mi, jlo * 128:512])
                            for h2, ot in ((0, ot0), (1, ot1)):
                                nc.tensor.matmul(
                                    ot[:, jlo * 128:512],
                                    lhsT=vone_sb[:, kv, kt, :],
                                    rhs=pt[:, h2, jlo * 128:512],
                                    start=(kt == kts[0][0]),
                                    stop=(kt == last_kt))
                        # normalize: denom (replicated on partitions 0:64,
                        # partition 0 in particular) -> fast recip -> gpsimd
                        # broadcast from partition 0 -> DVE multiply on the
                        # O half (partitions 64:128, all base-aligned).
                        # Even head writes otn_t[64:128] directly; odd head
                        # goes via SBUF->SBUF DMA to otn_t[0:64].
                        for h2, ot in ((0, ot0), (1, ot1)):
                            rc = rcp.tile([128, 512], f32, tag="rc")
                            nc.vector.reciprocal_approx_fast(
                                out=rc[0:1, :], in_=ot[0:1, :])
                            rbs = rcp.tile([128, 512], f32, tag="rbs")
                            nc.gpsimd.partition_broadcast(rbs[:], rc[0:1, :])
                            if h2 == 0:
                                nc.vector.tensor_mul(otn_t[64:128, p, :],
                                                     ot[64:128, :],
                                                     rbs[64:128, :])
                            else:
                                otmp = rcp.tile([128, 512], bf, tag="otmp")
                                nc.vector.tensor_mul(otmp[64:128, :],
                                                     ot[64:128, :],
                                                     rbs[64:128, :])
                                nc.sync.dma_start(otn_t[0:64, p, :],
                                                  otmp[64:128, :])
                    # wo for this q-stripe
                    for j in range(4):
                        sq0 = q0 + j * 128
                        for nb in range(4):
                            wp = psw2.tile([128, 512], f32)
                            for p in range(NPAIR):
                                nc.tensor.matmul(
                                    wp[:],
                                    lhsT=otn_t[:, p, j * 128:(j + 1) * 128],
                                    rhs=wo_sb[:, p, nb * 512:(nb + 1) * 512],
                                    start=(p == 0), stop=(p == NPAIR - 1))
                            wsb = wsp.tile([128, 512], bf)
                            if (j + nb) % 2 == 0:
                                nc.vector.tensor_copy(wsb[:], wp[:])
                            else:
                                nc.scalar.copy(wsb[:], wp[:])
                            nc.sync.dma_start(
                                out.ap()[sq0:sq0 + 128, nb * 512:(nb + 1) * 512],
                                wsb[:])
    nc.compile()
    return nc


def kernel(x, wq, wk, wv, wo, freqs, mask, start_pos):
    sys.path.insert(0, "/opt/trn_rl_repo")
    from concourse.bass_utils import run_bass_kernel_spmd

    x = np.asarray(x, dtype=np.float32)
    per_core, sched, U = _host_prepare(
        x, np.asarray(wq, np.float32), np.asarray(wk, np.float32),
        np.asarray(wv, np.float32), np.asarray(wo, np.float32),
        np.asarray(freqs, np.float32), np.asarray(mask, np.float32))

    nc = _build_program(sched, U)

    trace = bool(int(os.environ.get("BASSKERNEL_TRACE", "0")))
    if trace and "antenv.axon_hooks" not in sys.modules:
        # profile-hook shim (the trimmed antenv package lacks axon_hooks)
        try:
            import types

            if "/root/.axon_site" not in sys.path:
                sys.path.insert(0, "/root/.axon_site")
            from trn_agent_boot.trn_boot import _ntff_profile_via_ctypes

            _hook = _ntff_profile_via_ctypes("/opt/axon/libaxon_pjrt.so")
            _mod = types.ModuleType("antenv.axon_hooks")
            _mod.get_axon_ntff_profile_hook = lambda: _hook
            _mod.set_axon_ntff_profile_hook = lambda h: None
            sys.modules["antenv.axon_hooks"] = _mod
        except Exception:
            trace = False
    res = run_bass_kernel_spmd(nc, per_core, core_ids=list(range(NCORES)),
                               trace=trace)
    if trace:
        kernel._last_exec_time_ns = res.exec_time_ns
        kernel._last_profile = res.profile_json
    acc = np.zeros((B, S, D), np.float64)
    for c in range(NCORES):
        acc[c % 2] += res.results[c]["out"].astype(np.float64)
    return acc.astype(np.float32)


# revision 27
# speedup vs baseline: 1.2814x; 1.0411x over previous
"""GQA prefill attention (B=2, S=2048, D=2048, H=32, KV=8, HD=64) on 8 trn2 cores.

Sharding: batch x head-group. Core c owns batch b=c%2 and head-group hg=c//2:
q-heads [8hg, 8hg+8), kv-heads {2hg, 2hg+1}; computes its partial of
out[b] = attn_out @ wo_hg; host sums the 4 partials per batch.

Device kernel (per core, bf16 matmuls / fp32 PSUM):
  QT[dh,s] per head-pair (two 64-row heads stacked on 128 partitions)
  KT[dh,s] per kv head, duplicated onto both partition halves (row tiling)
  V[k,dh] natural layout via PE transpose; 65th column of ones
  RoPE via pair-swap permutation matmul + elementwise cos/sin tables
  ST[k,q] = KT^T @ QT for a head pair: two concurrent row-tiled K=64 matmuls
  P = exp(ST/8) one ScalarE instr per (pair, kt) over both PSUM banks
  OT[dh|sum, q] += [V | 1]^T-chunks @ P  (N=512 streaming PV, no LDW churn)
  normalize: recip(denom row) -> PE ones-broadcast -> DVE multiply
  out_partial[s,:] = OTn-chunks^T @ wo_hg   (bf16 partial to HBM)
"""

import os
import sys

import numpy as np
import ml_dtypes

BF16 = ml_dtypes.bfloat16

B, S, D, H, KV, HD = 2, 2048, 2048, 32, 8, 64
NCORES = 8
HPC = 8            # q-heads per core
NPAIR = 4          # head pairs per core
QS_TILES = S // 512
KT_TILES = S // 128
DC = D // 128      # contraction chunks for the projections

# per-pair wo row order: odd head first (otn_t partitions 0:64), even second
_WO_PERM = np.concatenate(
    [np.concatenate([np.arange((2 * p + 1) * 64, (2 * p + 2) * 64),
                     np.arange(2 * p * 64, (2 * p + 1) * 64)])
     for p in range(4)])


def _host_prepare(x, wq, wk, wv, wo, freqs, mask):
    """Build per-core device inputs + the mask block schedule."""
    # RoPE tables in the [dh-on-partitions, s] layout used by QT/KT.
    # Two 64-row head copies stacked (head pairs live on 128 partitions).
    # rope: out[2j]   = t[2j] cos - t[2j+1] sin
    #       out[2j+1] = t[2j] sin + t[2j+1] cos
    # with swap(t)[d] = t[d^1]:  out[d] = t[d]*cos[d] + swap(t)[d]*sgn(d)*sin[d]
    c64 = np.cos(freqs.T).repeat(2, axis=0).astype(np.float64)  # [64, S]
    s64 = np.sin(freqs.T).repeat(2, axis=0).astype(np.float64)
    sgn = np.where(np.arange(HD) % 2 == 0, -1.0, 1.0)[:, None]
    cos_t = np.concatenate([c64, c64], axis=0).astype(BF16)           # [128, S]
    sin_t = np.concatenate([s64 * sgn, s64 * sgn], axis=0).astype(BF16)

    # Mask block schedule at [128 k x 512 q] granularity (same for all b, h).
    # Block (qs, kt): full (mask all zero), skip (all <= -30), or masked
    # (multiply exp'd P by exp(mask^T) tile). jlo = first visible 128-q
    # subblock (clean fully-masked prefix only).
    mt_tiles = []  # unique [128, 512] multiplier tiles
    mt_keys = {}
    sched = []  # per qs: list of (kt, mtile_idx | None, jlo)
    for qs in range(QS_TILES):
        lst = []
        for kt in range(KT_TILES):
            blk = mask[qs * 512:(qs + 1) * 512, kt * 128:(kt + 1) * 128]  # [q, k]
            if np.all(blk <= -30.0):
                continue
            jmasked = [np.all(blk[j * 128:(j + 1) * 128] <= -30.0) for j in range(4)]
            jlo = 0
            while jlo < 4 and jmasked[jlo]:
                jlo += 1
            if any(jmasked[jlo:]):
                jlo = 0
            vis = blk[jlo * 128:]
            if np.all(vis == 0.0):
                lst.append((kt, None, jlo))
                continue
            tile_np = np.exp(blk.T.astype(np.float64)).astype(BF16)  # [128k, 512q]
            key = tile_np.tobytes()
            if key not in mt_keys:
                mt_keys[key] = len(mt_tiles)
                mt_tiles.append(tile_np)
            lst.append((kt, mt_keys[key], jlo))
        # every 128-q subblock needs at least one full-width contributing kt
        # so its softmax denominator is well-defined; also the FIRST kt per
        # subblock must be full-width (start=True clears the whole OT bank)
        for j in range(4):
            if not any(e[2] <= j for e in lst):
                lst = [(kt, mi, 0) for (kt, mi, _) in lst]
                break
        if lst and lst[0][2] != 0:
            lst[0] = (lst[0][0], lst[0][1], 0)
        sched.append(lst)
    if not mt_tiles:  # keep the input well-formed even if no masked blocks
        mt_tiles.append(np.ones((128, 512), dtype=BF16))
    mt = np.stack(mt_tiles)  # [U, 128, 512]

    per_core = []
    for c in range(NCORES):
        b, hg = c % 2, c // 2
        per_core.append({
            "xT": np.ascontiguousarray(x[b].T).astype(BF16),            # [D, S]
            "wq": np.ascontiguousarray(
                wq[:, hg * 512:(hg + 1) * 512]).astype(BF16),           # [D, 512]
            "wkv": np.ascontiguousarray(np.concatenate(
                [wk[:, hg * 128:(hg + 1) * 128],
                 wv[:, hg * 128:(hg + 1) * 128]], axis=1)).astype(BF16),  # [D, 256]
            # wo rows permuted to match otn_t layout: pair p holds head 2p+1
            # on partitions 0:64 and head 2p on partitions 64:128
            "wo": np.ascontiguousarray(
                wo[hg * 512:(hg + 1) * 512, :][_WO_PERM, :]).astype(BF16),  # [512, D]
            "cos": cos_t,
            "sin": sin_t,
            "mt": mt,
        })
    return per_core, sched, mt.shape[0]


def _build_program(sched, U):
    import concourse.bass as bass
    import concourse.mybir as mybir
    import concourse.tile as tile
    from concourse import bacc

    dt = mybir.dt
    bf, f32 = dt.bfloat16, dt.float32
    AF = mybir.ActivationFunctionType

    nc = bacc.Bacc("TRN2", target_bir_lowering=False, debug=False,
                   num_devices=NCORES)

    xT = nc.dram_tensor("xT", [D, S], bf, kind="ExternalInput")
    wq = nc.dram_tensor("wq", [D, 512], bf, kind="ExternalInput")
    wkv = nc.dram_tensor("wkv", [D, 256], bf, kind="ExternalInput")
    wo = nc.dram_tensor("wo", [512, D], bf, kind="ExternalInput")
    cos = nc.dram_tensor("cos", [128, S], bf, kind="ExternalInput")
    sin = nc.dram_tensor("sin", [128, S], bf, kind="ExternalInput")
    mt = nc.dram_tensor("mt", [U, 128, 512], bf, kind="ExternalInput")
    out = nc.dram_tensor("out", [S, D], bf, kind="ExternalOutput")

    # pair-swap permutation (block-diag over the two stacked 64-row heads)
    perm_np = np.zeros((128, 128), dtype=BF16)
    for d in range(128):
        perm_np[d ^ 1, d] = 1
    perm_dram = nc.inline_tensor(np.ascontiguousarray(perm_np), name="perm")
    ident_dram = nc.inline_tensor(np.eye(128, dtype=BF16), name="ident")

    with tile.TileContext(nc) as tc:
        with tc.tile_pool(name="const", bufs=1) as cp:
            # weight loads split per contraction chunk so the first projection
            # matmuls can start as soon as chunk 0 lands
            wq_sb = cp.tile([128, DC, 512], bf)
            wkv_sb = cp.tile([128, DC, 256], bf)
            wq_r = wq.ap().rearrange("(c p) m -> p c m", p=128)
            wkv_r = wkv.ap().rearrange("(c p) m -> p c m", p=128)
            for dc in range(DC):
                nc.sync.dma_start(wq_sb[:, dc, :], wq_r[:, dc, :])
                nc.sync.dma_start(wkv_sb[:, dc, :], wkv_r[:, dc, :])
            perm_sb = cp.tile([128, 128], bf)
            nc.sync.dma_start(perm_sb[:], perm_dram.ap())
            ident_sb = cp.tile([128, 128], bf)
            nc.sync.dma_start(ident_sb[:], ident_dram.ap())
            cos_sb = cp.tile([128, S], bf)
            nc.sync.dma_start(cos_sb[:], cos.ap())
            sin_sb = cp.tile([128, S], bf)
            nc.sync.dma_start(sin_sb[:], sin.ap())
            mt_sb = cp.tile([128, U, 512], bf)
            nc.sync.dma_start(mt_sb[:], mt.ap().rearrange("u p q -> p u q"))
            wo_sb = cp.tile([128, NPAIR, D], bf)
            nc.sync.dma_start(wo_sb[:], wo.ap().rearrange("(g p) n -> p g n", p=128))
            qt_sb = cp.tile([128, NPAIR, S], bf)    # [dh-pair, pair, s]
            # K^T zero-padded to a full 128 contraction so score matmuls stay
            # in the default 128x128 PE mode (no tiling-mode drains): slot 0
            # = [K | 0] (even head of the pair), slot 1 = [0 | K] (odd head)
            kt_sb = cp.tile([128, 2, 2, S], bf)     # [dh|0 halves, kv, slot, s]
            nc.vector.memset(kt_sb[64:128, :, 0, :], 0.0)
            nc.vector.memset(kt_sb[0:64, :, 1, :], 0.0)
            # PV stationary operand per (kv, kt): [ones(64) | V(64)] so the
            # softmax denominator lands on PSUM partitions 0:64 (partition 0
            # feeds the gpsimd broadcast) and O^T on partitions 64:128
            vone_sb = cp.tile([128, 2, KT_TILES, 128], bf)  # [k%128, kv, kt, 1|dh]
            nc.vector.memset(vone_sb[:, :, :, 0:HD], 1.0)

            # ---------------- phase 1: projections + rope ----------------
            with (
                tc.tile_pool(name="xt", bufs=3) as xp,
                tc.tile_pool(name="raw", bufs=2) as rawp,
                tc.tile_pool(name="rtmp", bufs=2) as rtp,
                tc.tile_pool(name="ps_pr", bufs=1, space="PSUM") as ppr,
                tc.tile_pool(name="ps_sw", bufs=1, space="PSUM") as psw,
                tc.tile_pool(name="ps_vt", bufs=1, space="PSUM") as pvt,
            ):
                xT_r = xT.ap().rearrange("(c p) s -> p c s", p=128)
                for st in range(S // 512):
                    s0 = st * 512
                    xbig = xp.tile([128, DC, 512], bf)
                    # split the 2MB chunk load across DMA queues
                    for dc2 in range(0, DC, 2):
                        nc.sync.dma_start(xbig[:, dc2:dc2 + 2, :],
                                          xT_r[:, dc2:dc2 + 2, s0:s0 + 512])
                    qps = [ppr.tile([128, 512], f32, tag=f"q{i}", name=f"qp{i}")
                           for i in range(NPAIR)]
                    kp = ppr.tile([128, 512], f32, tag="kp")
                    vp = ppr.tile([128, 512], f32, tag="vp")
                    for dc in range(DC):
                        st_, sp_ = (dc == 0), (dc == DC - 1)
                        for i in range(NPAIR):
                            nc.tensor.matmul(qps[i][:],
                                             lhsT=wq_sb[:, dc, i * 128:(i + 1) * 128],
                                             rhs=xbig[:, dc, :], start=st_, stop=sp_)
                        nc.tensor.matmul(kp[:], lhsT=wkv_sb[:, dc, 0:128],
                                         rhs=xbig[:, dc, :], start=st_, stop=sp_)
                        nc.tensor.matmul(vp[:], lhsT=wkv_sb[:, dc, 128:256],
                                         rhs=xbig[:, dc, :], start=st_, stop=sp_)
                    # raw copies to SBUF (also the swap-matmul inputs)
                    qrs = []
                    for i in range(NPAIR):
                        qr = rawp.tile([128, 512], bf, tag=f"q{i}r", name=f"q{i}r")
                        nc.vector.tensor_copy(qr[:], qps[i][:])
                        qrs.append(qr)
                    kr = rawp.tile([128, 512], bf, tag="kr")
                    nc.scalar.copy(kr[:], kp[:])
                    vr = rawp.tile([128, 512], bf, tag="vr")
                    nc.scalar.copy(vr[:], vp[:])
                    # V: transpose VT rows back to natural [k, dh]
                    for kv in range(2):
                        pl = kv * 64
                        for j in range(4):
                            vtp = pvt.tile([128, HD], bf)
                            nc.tensor.transpose(
                                vtp[:], vr[pl:pl + 64, j * 128:(j + 1) * 128],
                                ident_sb[pl:pl + 64, pl:pl + 64])
                            nc.vector.tensor_copy(
                                vone_sb[:, kv, 4 * st + j, HD:128], vtp[:])
                    # rope Q (pairs stacked on 128 partitions)
                    for i, qr in enumerate(qrs):
                        swp = psw.tile([128, 512], f32, tag="sw")
                        nc.tensor.matmul(swp[:], lhsT=perm_sb[:], rhs=qr[:],
                                         start=True, stop=True)
                        t_sin = rtp.tile([128, 512], bf, tag="tsin")
                        nc.vector.tensor_mul(t_sin[:], swp[:], sin_sb[:, s0:s0 + 512])
                        t_cos = rtp.tile([128, 512], bf, tag="tcos")
                        nc.vector.tensor_mul(t_cos[:], qr[:], cos_sb[:, s0:s0 + 512])
                        nc.vector.tensor_add(qt_sb[:, i, s0:s0 + 512],
                                             t_sin[:], t_cos[:])
                    # rope K (both kv heads stacked); write into the dup'd
                    # halves, then DMA-duplicate across partition halves
                    ksw = psw.tile([128, 512], f32, tag="sw")
                    nc.tensor.matmul(ksw[:], lhsT=perm_sb[:], rhs=kr[:],
                                     start=True, stop=True)
                    k_sin = rtp.tile([128, 512], bf, tag="tsin")
                    nc.vector.tensor_mul(k_sin[:], ksw[:], sin_sb[:, s0:s0 + 512])
                    k_cos = rtp.tile([128, 512], bf, tag="tcos")
                    nc.vector.tensor_mul(k_cos[:], kr[:], cos_sb[:, s0:s0 + 512])
                    nc.vector.tensor_add(kt_sb[0:64, 0, 0, s0:s0 + 512],
                                         k_sin[0:64, :], k_cos[0:64, :])
                    nc.vector.tensor_add(kt_sb[64:128, 1, 1, s0:s0 + 512],
                                         k_sin[64:128, :], k_cos[64:128, :])
                    nc.sync.dma_start(kt_sb[64:128, 0, 1, s0:s0 + 512],
                                      kt_sb[0:64, 0, 0, s0:s0 + 512])
                    nc.sync.dma_start(kt_sb[0:64, 1, 0, s0:s0 + 512],
                                      kt_sb[64:128, 1, 1, s0:s0 + 512])

            # ---------------- phase 2: attention + wo ----------------
            with (
                tc.tile_pool(name="pp", bufs=3) as ppool,
                tc.tile_pool(name="rcp", bufs=2) as rcp,
                tc.tile_pool(name="otn", bufs=2) as otp,
                tc.tile_pool(name="wsb", bufs=3) as wsp,
                tc.tile_pool(name="ps_s", bufs=2, space="PSUM") as pss,
                tc.tile_pool(name="ps_o", bufs=1, space="PSUM") as pso,
                tc.tile_pool(name="ps_w", bufs=2, space="PSUM") as psw2,
            ):
                for qs in range(QS_TILES):
                    q0 = qs * 512
                    kts = sched[qs]
                    last_kt = max(e[0] for e in kts)
                    otn_t = otp.tile([128, NPAIR, 512], bf)  # [dh-pair, pair, q]
                    for p in range(NPAIR):
                        kv = p // 2
                        ot0 = pso.tile([128, 512], f32, tag="ot0", name="ot0")
                        ot1 = pso.tile([128, 512], f32, tag="ot1", name="ot1")
                        for kt, mi, jlo in kts:
                            nq = 512 - jlo * 128
                            ql = q0 + jlo * 128
                            spp = pss.tile([128, 2, 512], f32)
                            for h2 in range(2):
                                nc.tensor.matmul(
                                    spp[:, h2, jlo * 128:512],
                                    lhsT=kt_sb[:, kv, h2,
                                               kt * 128:(kt + 1) * 128],
                                    rhs=qt_sb[:, p, ql:q0 + 512],
                                    start=True, stop=True)
                            pt = ppool.tile([128, 2, 512], bf)
                            nc.scalar.activation(pt[:, :, jlo * 128:512],
                                                 spp[:, :, jlo * 128:512], AF.Exp,
                                                 scale=1.0 / np.sqrt(HD))
                            if mi is not None:
                                for h2 in range(2):
                                    nc.vector.tensor_mul(
                                        pt[:, h2, jlo * 128:512],
                                        pt[:, h2, jlo * 128:512],
                                        mt_sb[:, mi, jlo * 128:512])
                            for h2, ot in ((0, ot0), (1, ot1)):
                                nc.tensor.matmul(
                                    ot[:, jlo * 128:512],
                                    lhsT=vone_sb[:, kv, kt, :],
                                    rhs=pt[:, h2, jlo * 128:512],
                                    start=(kt == kts[0][0]),
                                    stop=(kt == last_kt))
                        # normalize: denom (replicated on partitions 0:64,
                        # partition 0 in particular) -> fast recip -> gpsimd
                        # broadcast from partition 0 -> DVE multiply on the
                        # O half (partitions 64:128, all base-aligned).
                        # Even head writes otn_t[64:128] directly; odd head
                        # goes via SBUF->SBUF DMA to otn_t[0:64].
                        for h2, ot in ((0, ot0), (1, ot1)):
                            rc = rcp.tile([128, 512], f32, tag="rc")
                            nc.vector.reciprocal_approx_fast(
                                out=rc[0:1, :], in_=ot[0:1, :])
                            rbs = rcp.tile([128, 512], f32, tag="rbs")
                            nc.gpsimd.partition_broadcast(rbs[:], rc[0:1, :])
                            if h2 == 0:
                                nc.vector.tensor_mul(otn_t[64:128, p, :],
                                                     ot[64:128, :],
                                                     rbs[64:128, :])
                            else:
                                otmp = rcp.tile([128, 512], bf, tag="otmp")
                                nc.vector.tensor_mul(otmp[64:128, :],
                                                     ot[64:128, :],
                                                     rbs[64:128, :])
                                nc.sync.dma_start(otn_t[0:64, p, :],
                                                  otmp[64:128, :])
                    # wo for this q-stripe
                    for j in range(4):
                        sq0 = q0 + j * 128
                        for nb in range(4):
                            wp = psw2.tile([128, 512], f32)
                            for p in range(NPAIR):
                                nc.tensor.matmul(
                                    wp[:],
                                    lhsT=otn_t[:, p, j * 128:(j + 1) * 128],
                                    rhs=wo_sb[:, p, nb * 512:(nb + 1) * 512],
                                    start=(p == 0), stop=(p == NPAIR - 1))
                            wsb = wsp.tile([128, 512], bf)
                            if (j + nb) % 2 == 0:
                                nc.vector.tensor_copy(wsb[:], wp[:])
                            else:
                                nc.scalar.copy(wsb[:], wp[:])
                            nc.sync.dma_start(
                                out.ap()[sq0:sq0 + 128, nb * 512:(nb + 1) * 512],
                                wsb[:])
    nc.compile()
    return nc


def kernel(x, wq, wk, wv, wo, freqs, mask, start_pos):
    sys.path.insert(0, "/opt/trn_rl_repo")
    from concourse.bass_utils import run_bass_kernel_spmd

    x = np.asarray(x, dtype=np.float32)
    per_core, sched, U = _host_prepare(
        x, np.asarray(wq, np.float32), np.asarray(wk, np.float32),
        np.asarray(wv, np.float32), np.asarray(wo, np.float32),
        np.asarray(freqs, np.float32), np.asarray(mask, np.float32))

    nc = _build_program(sched, U)

    trace = bool(int(os.environ.get("BASSKERNEL_TRACE", "0")))
    if trace and "antenv.axon_hooks" not in sys.modules:
        # profile-hook shim (the trimmed antenv package lacks axon_hooks)
        try:
            import types

            if "/root/.axon_site" not in sys.path:
                sys.path.insert(0, "/root/.axon_site")
            from trn_agent_boot.trn_boot import _ntff_profile_via_ctypes

            _hook = _ntff_profile_via_ctypes("/opt/axon/libaxon_pjrt.so")
            _mod = types.ModuleType("antenv.axon_hooks")
            _mod.get_axon_ntff_profile_hook = lambda: _hook
            _mod.set_axon_ntff_profile_hook = lambda h: None
            sys.modules["antenv.axon_hooks"] = _mod
        except Exception:
            trace = False
    res = run_bass_kernel_spmd(nc, per_core, core_ids=list(range(NCORES)),
                               trace=trace)
    if trace:
        kernel._last_exec_time_ns = res.exec_time_ns
        kernel._last_profile = res.profile_json
    acc = np.zeros((B, S, D), np.float64)
    for c in range(NCORES):
        acc[c % 2] += res.results[c]["out"].astype(np.float64)
    return acc.astype(np.float32)


# revision 30
# speedup vs baseline: 1.3121x; 1.0240x over previous
"""GQA prefill attention (B=2, S=2048, D=2048, H=32, KV=8, HD=64) on 8 trn2 cores.

Sharding: batch x head-group. Core c owns batch b=c%2 and head-group hg=c//2:
q-heads [8hg, 8hg+8), kv-heads {2hg, 2hg+1}; computes its partial of
out[b] = attn_out @ wo_hg; host sums the 4 partials per batch.

Device kernel (per core, bf16 matmuls / fp32 PSUM):
  QT[dh,s] per head-pair (two 64-row heads stacked on 128 partitions)
  KT[dh,s] per kv head, duplicated onto both partition halves (row tiling)
  V[k,dh] natural layout via PE transpose; 65th column of ones
  RoPE via pair-swap permutation matmul + elementwise cos/sin tables
  ST[k,q] = KT^T @ QT for a head pair: two concurrent row-tiled K=64 matmuls
  P = exp(ST/8) one ScalarE instr per (pair, kt) over both PSUM banks
  OT[dh|sum, q] += [V | 1]^T-chunks @ P  (N=512 streaming PV, no LDW churn)
  normalize: recip(denom row) -> PE ones-broadcast -> DVE multiply
  out_partial[s,:] = OTn-chunks^T @ wo_hg   (bf16 partial to HBM)
"""

import os
import sys

import numpy as np
import ml_dtypes

BF16 = ml_dtypes.bfloat16

B, S, D, H, KV, HD = 2, 2048, 2048, 32, 8, 64
NCORES = 8
HPC = 8            # q-heads per core
NPAIR = 4          # head pairs per core
QS_TILES = S // 512
KT_TILES = S // 128
DC = D // 128      # contraction chunks for the projections

# per-pair wo row order: odd head first (otn_t partitions 0:64), even second
_WO_PERM = np.concatenate(
    [np.concatenate([np.arange((2 * p + 1) * 64, (2 * p + 2) * 64),
                     np.arange(2 * p * 64, (2 * p + 1) * 64)])
     for p in range(4)])


def _host_prepare(x, wq, wk, wv, wo, freqs, mask):
    """Build per-core device inputs + the mask block schedule."""
    # RoPE tables in the [dh-on-partitions, s] layout used by QT/KT.
    # Two 64-row head copies stacked (head pairs live on 128 partitions).
    # rope: out[2j]   = t[2j] cos - t[2j+1] sin
    #       out[2j+1] = t[2j] sin + t[2j+1] cos
    # with swap(t)[d] = t[d^1]:  out[d] = t[d]*cos[d] + swap(t)[d]*sgn(d)*sin[d]
    c64 = np.cos(freqs.T).repeat(2, axis=0).astype(np.float64)  # [64, S]
    s64 = np.sin(freqs.T).repeat(2, axis=0).astype(np.float64)
    sgn = np.where(np.arange(HD) % 2 == 0, -1.0, 1.0)[:, None]
    cos_t = np.concatenate([c64, c64], axis=0).astype(BF16)           # [128, S]
    sin_t = np.concatenate([s64 * sgn, s64 * sgn], axis=0).astype(BF16)

    # Mask block schedule at [128 k x 512 q] granularity (same for all b, h).
    # Block (qs, kt): full (mask all zero), skip (all <= -30), or masked
    # (multiply exp'd P by exp(mask^T) tile). jlo = first visible 128-q
    # subblock (clean fully-masked prefix only).
    mt_tiles = []  # unique [128, 512] multiplier tiles
    mt_keys = {}
    sched = []  # per qs: list of (kt, mtile_idx | None, jlo)
    for qs in range(QS_TILES):
        lst = []
        for kt in range(KT_TILES):
            blk = mask[qs * 512:(qs + 1) * 512, kt * 128:(kt + 1) * 128]  # [q, k]
            if np.all(blk <= -30.0):
                continue
            jmasked = [np.all(blk[j * 128:(j + 1) * 128] <= -30.0) for j in range(4)]
            jlo = 0
            while jlo < 4 and jmasked[jlo]:
                jlo += 1
            if any(jmasked[jlo:]):
                jlo = 0
            vis = blk[jlo * 128:]
            if np.all(vis == 0.0):
                lst.append((kt, None, jlo))
                continue
            tile_np = np.exp(blk.T.astype(np.float64)).astype(BF16)  # [128k, 512q]
            key = tile_np.tobytes()
            if key not in mt_keys:
                mt_keys[key] = len(mt_tiles)
                mt_tiles.append(tile_np)
            lst.append((kt, mt_keys[key], jlo))
        # every 128-q subblock needs at least one full-width contributing kt
        # so its softmax denominator is well-defined; also the FIRST kt per
        # subblock must be full-width (start=True clears the whole OT bank)
        for j in range(4):
            if not any(e[2] <= j for e in lst):
                lst = [(kt, mi, 0) for (kt, mi, _) in lst]
                break
        if lst and lst[0][2] != 0:
            lst[0] = (lst[0][0], lst[0][1], 0)
        sched.append(lst)
    if not mt_tiles:  # keep the input well-formed even if no masked blocks
        mt_tiles.append(np.ones((128, 512), dtype=BF16))
    mt = np.stack(mt_tiles)  # [U, 128, 512]

    per_core = []
    for c in range(NCORES):
        b, hg = c % 2, c // 2
        per_core.append({
            "xT": np.ascontiguousarray(x[b].T).astype(BF16),            # [D, S]
            "wq": np.ascontiguousarray(
                wq[:, hg * 512:(hg + 1) * 512]).astype(BF16),           # [D, 512]
            "wkv": np.ascontiguousarray(np.concatenate(
                [wk[:, hg * 128:(hg + 1) * 128],
                 wv[:, hg * 128:(hg + 1) * 128]], axis=1)).astype(BF16),  # [D, 256]
            # wo rows permuted to match otn_t layout: pair p holds head 2p+1
            # on partitions 0:64 and head 2p on partitions 64:128
            "wo": np.ascontiguousarray(
                wo[hg * 512:(hg + 1) * 512, :][_WO_PERM, :]).astype(BF16),  # [512, D]
            "cos": cos_t,
            "sin": sin_t,
            "mt": mt,
        })
    return per_core, sched, mt.shape[0]


def _build_program(sched, U):
    import concourse.bass as bass
    import concourse.mybir as mybir
    import concourse.tile as tile
    from concourse import bacc

    dt = mybir.dt
    bf, f32 = dt.bfloat16, dt.float32
    AF = mybir.ActivationFunctionType

    nc = bacc.Bacc("TRN2", target_bir_lowering=False, debug=False,
                   num_devices=NCORES)

    xT = nc.dram_tensor("xT", [D, S], bf, kind="ExternalInput")
    wq = nc.dram_tensor("wq", [D, 512], bf, kind="ExternalInput")
    wkv = nc.dram_tensor("wkv", [D, 256], bf, kind="ExternalInput")
    wo = nc.dram_tensor("wo", [512, D], bf, kind="ExternalInput")
    cos = nc.dram_tensor("cos", [128, S], bf, kind="ExternalInput")
    sin = nc.dram_tensor("sin", [128, S], bf, kind="ExternalInput")
    mt = nc.dram_tensor("mt", [U, 128, 512], bf, kind="ExternalInput")
    out = nc.dram_tensor("out", [S, D], bf, kind="ExternalOutput")

    # pair-swap permutation (block-diag over the two stacked 64-row heads)
    perm_np = np.zeros((128, 128), dtype=BF16)
    for d in range(128):
        perm_np[d ^ 1, d] = 1
    perm_dram = nc.inline_tensor(np.ascontiguousarray(perm_np), name="perm")
    ident_dram = nc.inline_tensor(np.eye(128, dtype=BF16), name="ident")

    with tile.TileContext(nc) as tc:
        with tc.tile_pool(name="const", bufs=1) as cp:
            # DMA issue order matters: the first projection matmuls need
            # x chunk 0 + the weight chunks, so those go first; everything
            # needed later (wo, mask tiles, rope tables) queues after.
            wq_sb = cp.tile([128, DC, 512], bf)
            wkv_sb = cp.tile([128, DC, 256], bf)
            xbig0 = cp.tile([128, DC, 512], bf, name="xbig0")
            wq_r = wq.ap().rearrange("(c p) m -> p c m", p=128)
            wkv_r = wkv.ap().rearrange("(c p) m -> p c m", p=128)
            xT_r = xT.ap().rearrange("(c p) s -> p c s", p=128)
            for dc in range(DC):
                nc.sync.dma_start(xbig0[:, dc, :], xT_r[:, dc, 0:512])
                nc.sync.dma_start(wq_sb[:, dc, :], wq_r[:, dc, :])
                nc.sync.dma_start(wkv_sb[:, dc, :], wkv_r[:, dc, :])
            perm_sb = cp.tile([128, 128], bf)
            nc.sync.dma_start(perm_sb[:], perm_dram.ap())
            ident_sb = cp.tile([128, 128], bf)
            nc.sync.dma_start(ident_sb[:], ident_dram.ap())
            cos_sb = cp.tile([128, S], bf)
            nc.sync.dma_start(cos_sb[:], cos.ap())
            sin_sb = cp.tile([128, S], bf)
            nc.sync.dma_start(sin_sb[:], sin.ap())
            mt_sb = cp.tile([128, U, 512], bf)
            nc.sync.dma_start(mt_sb[:], mt.ap().rearrange("u p q -> p u q"))
            wo_sb = cp.tile([128, NPAIR, D], bf)
            nc.sync.dma_start(wo_sb[:], wo.ap().rearrange("(g p) n -> p g n", p=128))
            qt_sb = cp.tile([128, NPAIR, S], bf)    # [dh-pair, pair, s]
            # K^T zero-padded to a full 128 contraction so score matmuls stay
            # in the default 128x128 PE mode (no tiling-mode drains): slot 0
            # = [K | 0] (even head of the pair), slot 1 = [0 | K] (odd head)
            kt_sb = cp.tile([128, 2, 2, S], bf)     # [dh|0 halves, kv, slot, s]
            nc.vector.memset(kt_sb[64:128, :, 0, :], 0.0)
            nc.vector.memset(kt_sb[0:64, :, 1, :], 0.0)
            # PV stationary operand per (kv, kt): [ones(64) | V(64)] so the
            # softmax denominator lands on PSUM partitions 0:64 (partition 0
            # feeds the gpsimd broadcast) and O^T on partitions 64:128
            vone_sb = cp.tile([128, 2, KT_TILES, 128], bf)  # [k%128, kv, kt, 1|dh]
            nc.vector.memset(vone_sb[:, :, :, 0:HD], 1.0)

            # ---------------- phase 1: projections + rope ----------------
            with (
                tc.tile_pool(name="xt", bufs=3) as xp,
                tc.tile_pool(name="raw", bufs=2) as rawp,
                tc.tile_pool(name="rtmp", bufs=2) as rtp,
                tc.tile_pool(name="ps_pr", bufs=1, space="PSUM") as ppr,
                tc.tile_pool(name="ps_sw", bufs=1, space="PSUM") as psw,
                tc.tile_pool(name="ps_vt", bufs=1, space="PSUM") as pvt,
            ):
                for st in range(S // 512):
                    s0 = st * 512
                    if st == 0:
                        xbig = xbig0
                    else:
                        xbig = xp.tile([128, DC, 512], bf)
                        # split the 2MB chunk load across DMA queues
                        for dc2 in range(0, DC, 2):
                            nc.sync.dma_start(xbig[:, dc2:dc2 + 2, :],
                                              xT_r[:, dc2:dc2 + 2, s0:s0 + 512])
                    qps = [ppr.tile([128, 512], f32, tag=f"q{i}", name=f"qp{i}")
                           for i in range(NPAIR)]
                    kp = ppr.tile([128, 512], f32, tag="kp")
                    vp = ppr.tile([128, 512], f32, tag="vp")
                    for dc in range(DC):
                        st_, sp_ = (dc == 0), (dc == DC - 1)
                        for i in range(NPAIR):
                            nc.tensor.matmul(qps[i][:],
                                             lhsT=wq_sb[:, dc, i * 128:(i + 1) * 128],
                                             rhs=xbig[:, dc, :], start=st_, stop=sp_)
                        nc.tensor.matmul(kp[:], lhsT=wkv_sb[:, dc, 0:128],
                                         rhs=xbig[:, dc, :], start=st_, stop=sp_)
                        nc.tensor.matmul(vp[:], lhsT=wkv_sb[:, dc, 128:256],
                                         rhs=xbig[:, dc, :], start=st_, stop=sp_)
                    # raw copies to SBUF (also the swap-matmul inputs)
                    qrs = []
                    for i in range(NPAIR):
                        qr = rawp.tile([128, 512], bf, tag=f"q{i}r", name=f"q{i}r")
                        nc.vector.tensor_copy(qr[:], qps[i][:])
                        qrs.append(qr)
                    kr = rawp.tile([128, 512], bf, tag="kr")
                    nc.vector.tensor_copy(kr[:], kp[:])
                    vr = rawp.tile([128, 512], bf, tag="vr")
                    nc.vector.tensor_copy(vr[:], vp[:])
                    # V: transpose VT rows back to natural [k, dh]
                    for kv in range(2):
                        pl = kv * 64
                        for j in range(4):
                            vtp = pvt.tile([128, HD], bf)
                            nc.tensor.transpose(
                                vtp[:], vr[pl:pl + 64, j * 128:(j + 1) * 128],
                                ident_sb[pl:pl + 64, pl:pl + 64])
                            nc.vector.tensor_copy(
                                vone_sb[:, kv, 4 * st + j, HD:128], vtp[:])
                    # rope Q (pairs stacked on 128 partitions)
                    for i, qr in enumerate(qrs):
                        swp = psw.tile([128, 512], f32, tag="sw")
                        nc.tensor.matmul(swp[:], lhsT=perm_sb[:], rhs=qr[:],
                                         start=True, stop=True)
                        t_sin = rtp.tile([128, 512], bf, tag="tsin")
                        nc.vector.tensor_mul(t_sin[:], swp[:], sin_sb[:, s0:s0 + 512])
                        t_cos = rtp.tile([128, 512], bf, tag="tcos")
                        nc.vector.tensor_mul(t_cos[:], qr[:], cos_sb[:, s0:s0 + 512])
                        nc.vector.tensor_add(qt_sb[:, i, s0:s0 + 512],
                                             t_sin[:], t_cos[:])
                    # rope K (both kv heads stacked); write into the dup'd
                    # halves, then DMA-duplicate across partition halves
                    ksw = psw.tile([128, 512], f32, tag="sw")
                    nc.tensor.matmul(ksw[:], lhsT=perm_sb[:], rhs=kr[:],
                                     start=True, stop=True)
                    k_sin = rtp.tile([128, 512], bf, tag="tsin")
                    nc.vector.tensor_mul(k_sin[:], ksw[:], sin_sb[:, s0:s0 + 512])
                    k_cos = rtp.tile([128, 512], bf, tag="tcos")
                    nc.vector.tensor_mul(k_cos[:], kr[:], cos_sb[:, s0:s0 + 512])
                    nc.vector.tensor_add(kt_sb[0:64, 0, 0, s0:s0 + 512],
                                         k_sin[0:64, :], k_cos[0:64, :])
                    nc.vector.tensor_add(kt_sb[64:128, 1, 1, s0:s0 + 512],
                                         k_sin[64:128, :], k_cos[64:128, :])
                    nc.sync.dma_start(kt_sb[64:128, 0, 1, s0:s0 + 512],
                                      kt_sb[0:64, 0, 0, s0:s0 + 512])
                    nc.sync.dma_start(kt_sb[0:64, 1, 0, s0:s0 + 512],
                                      kt_sb[64:128, 1, 1, s0:s0 + 512])

            # ---------------- phase 2: attention + wo ----------------
            with (
                tc.tile_pool(name="pp", bufs=3) as ppool,
                tc.tile_pool(name="rcp", bufs=2) as rcp,
                tc.tile_pool(name="otn", bufs=2) as otp,
                tc.tile_pool(name="wsb", bufs=3) as wsp,
                tc.tile_pool(name="ps_s", bufs=2, space="PSUM") as pss,
                tc.tile_pool(name="ps_o", bufs=1, space="PSUM") as pso,
                tc.tile_pool(name="ps_w", bufs=2, space="PSUM") as psw2,
            ):
                for qs in range(QS_TILES):
                    q0 = qs * 512
                    kts = sched[qs]
                    last_kt = max(e[0] for e in kts)
                    otn_t = otp.tile([128, NPAIR, 512], bf)  # [dh-pair, pair, q]
                    for p in range(NPAIR):
                        kv = p // 2
                        ot0 = pso.tile([128, 512], f32, tag="ot0", name="ot0")
                        ot1 = pso.tile([128, 512], f32, tag="ot1", name="ot1")
                        for kt, mi, jlo in kts:
                            nq = 512 - jlo * 128
                            ql = q0 + jlo * 128
                            spp = pss.tile([128, 2, 512], f32)
                            for h2 in range(2):
                                nc.tensor.matmul(
                                    spp[:, h2, jlo * 128:512],
                                    lhsT=kt_sb[:, kv, h2,
                                               kt * 128:(kt + 1) * 128],
                                    rhs=qt_sb[:, p, ql:q0 + 512],
                                    start=True, stop=True)
                            pt = ppool.tile([128, 2, 512], bf)
                            nc.scalar.activation(pt[:, :, jlo * 128:512],
                                                 spp[:, :, jlo * 128:512], AF.Exp,
                                                 scale=1.0 / np.sqrt(HD))
                            if mi is not None:
                                for h2 in range(2):
                                    nc.vector.tensor_mul(
                                        pt[:, h2, jlo * 128:512],
                                        pt[:, h2, jlo * 128:512],
                                        mt_sb[:, mi, jlo * 128:512])
                            for h2, ot in ((0, ot0), (1, ot1)):
                                nc.tensor.matmul(
                                    ot[:, jlo * 128:512],
                                    lhsT=vone_sb[:, kv, kt, :],
                                    rhs=pt[:, h2, jlo * 128:512],
                                    start=(kt == kts[0][0]),
                                    stop=(kt == last_kt))
                        # normalize: denom (replicated on partitions 0:64,
                        # partition 0 in particular) -> fast recip -> gpsimd
                        # broadcast from partition 0 -> DVE multiply on the
                        # O half (partitions 64:128, all base-aligned).
                        # Even head writes otn_t[64:128] directly; odd head
                        # goes via SBUF->SBUF DMA to otn_t[0:64].
                        for h2, ot in ((0, ot0), (1, ot1)):
                            rc = rcp.tile([128, 512], f32, tag="rc")
                            nc.vector.reciprocal_approx_fast(
                                out=rc[0:1, :], in_=ot[0:1, :])
                            rbs = rcp.tile([128, 512], f32, tag="rbs")
                            nc.gpsimd.partition_broadcast(rbs[:], rc[0:1, :])
                            if h2 == 0:
                                nc.vector.tensor_mul(otn_t[64:128, p, :],
                                                     ot[64:128, :],
                                                     rbs[64:128, :])
                            else:
                                otmp = rcp.tile([128, 512], bf, tag="otmp")
                                nc.vector.tensor_mul(otmp[64:128, :],
                                                     ot[64:128, :],
                                                     rbs[64:128, :])
                                nc.sync.dma_start(otn_t[0:64, p, :],
                                                  otmp[64:128, :])
                    # wo for this q-stripe
                    for j in range(4):
                        sq0 = q0 + j * 128
                        for nb in range(4):
                            wp = psw2.tile([128, 512], f32)
                            for p in range(NPAIR):
                                nc.tensor.matmul(
                                    wp[:],
                                    lhsT=otn_t[:, p, j * 128:(j + 1) * 128],
                                    rhs=wo_sb[:, p, nb * 512:(nb + 1) * 512],
                                    start=(p == 0), stop=(p == NPAIR - 1))
                            wsb = wsp.tile([128, 512], bf)
                            if (j + nb) % 2 == 0:
                                nc.vector.tensor_copy(wsb[:], wp[:])
                            else:
                                nc.scalar.copy(wsb[:], wp[:])
                            nc.sync.dma_start(
                                out.ap()[sq0:sq0 + 128, nb * 512:(nb + 1) * 512],
                                wsb[:])
    nc.compile()
    return nc


def kernel(x, wq, wk, wv, wo, freqs, mask, start_pos):
    sys.path.insert(0, "/opt/trn_rl_repo")
    from concourse.bass_utils import run_bass_kernel_spmd

    x = np.asarray(x, dtype=np.float32)
    per_core, sched, U = _host_prepare(
        x, np.asarray(wq, np.float32), np.asarray(wk, np.float32),
        np.asarray(wv, np.float32), np.asarray(wo, np.float32),
        np.asarray(freqs, np.float32), np.asarray(mask, np.float32))

    nc = _build_program(sched, U)

    trace = bool(int(os.environ.get("BASSKERNEL_TRACE", "0")))
    if trace and "antenv.axon_hooks" not in sys.modules:
        # profile-hook shim (the trimmed antenv package lacks axon_hooks)
        try:
            import types

            if "/root/.axon_site" not in sys.path:
                sys.path.insert(0, "/root/.axon_site")
            from trn_agent_boot.trn_boot import _ntff_profile_via_ctypes

            _hook = _ntff_profile_via_ctypes("/opt/axon/libaxon_pjrt.so")
            _mod = types.ModuleType("antenv.axon_hooks")
            _mod.get_axon_ntff_profile_hook = lambda: _hook
            _mod.set_axon_ntff_profile_hook = lambda h: None
            sys.modules["antenv.axon_hooks"] = _mod
        except Exception:
            trace = False
    res = run_bass_kernel_spmd(nc, per_core, core_ids=list(range(NCORES)),
                               trace=trace)
    if trace:
        kernel._last_exec_time_ns = res.exec_time_ns
        kernel._last_profile = res.profile_json
    acc = np.zeros((B, S, D), np.float64)
    for c in range(NCORES):
        acc[c % 2] += res.results[c]["out"].astype(np.float64)
    return acc.astype(np.float32)


# revision 34
# speedup vs baseline: 1.3170x; 1.0037x over previous
"""GQA prefill attention (B=2, S=2048, D=2048, H=32, KV=8, HD=64) on 8 trn2 cores.

Sharding: batch x head-group. Core c owns batch b=c%2 and head-group hg=c//2:
q-heads [8hg, 8hg+8), kv-heads {2hg, 2hg+1}; computes its partial of
out[b] = attn_out @ wo_hg; host sums the 4 partials per batch.

Device kernel (per core, bf16 matmuls / fp32 PSUM):
  QT[dh,s] per head-pair (two 64-row heads stacked on 128 partitions)
  KT[dh,s] per kv head, duplicated onto both partition halves (row tiling)
  V[k,dh] natural layout via PE transpose; 65th column of ones
  RoPE via pair-swap permutation matmul + elementwise cos/sin tables
  ST[k,q] = KT^T @ QT for a head pair: two concurrent row-tiled K=64 matmuls
  P = exp(ST/8) one ScalarE instr per (pair, kt) over both PSUM banks
  OT[dh|sum, q] += [V | 1]^T-chunks @ P  (N=512 streaming PV, no LDW churn)
  normalize: recip(denom row) -> PE ones-broadcast -> DVE multiply
  out_partial[s,:] = OTn-chunks^T @ wo_hg   (bf16 partial to HBM)
"""

import os
import sys

import numpy as np
import ml_dtypes

BF16 = ml_dtypes.bfloat16

B, S, D, H, KV, HD = 2, 2048, 2048, 32, 8, 64
NCORES = 8
HPC = 8            # q-heads per core
NPAIR = 4          # head pairs per core
QS_TILES = S // 512
KT_TILES = S // 128
DC = D // 128      # contraction chunks for the projections

# per-pair wo row order: odd head first (otn_t partitions 0:64), even second
_WO_PERM = np.concatenate(
    [np.concatenate([np.arange((2 * p + 1) * 64, (2 * p + 2) * 64),
                     np.arange(2 * p * 64, (2 * p + 1) * 64)])
     for p in range(4)])


def _host_prepare(x, wq, wk, wv, wo, freqs, mask):
    """Build per-core device inputs + the mask block schedule."""
    # RoPE tables in the [dh-on-partitions, s] layout used by QT/KT.
    # Two 64-row head copies stacked (head pairs live on 128 partitions).
    # rope: out[2j]   = t[2j] cos - t[2j+1] sin
    #       out[2j+1] = t[2j] sin + t[2j+1] cos
    # with swap(t)[d] = t[d^1]:  out[d] = t[d]*cos[d] + swap(t)[d]*sgn(d)*sin[d]
    c64 = np.cos(freqs.T).repeat(2, axis=0).astype(np.float64)  # [64, S]
    s64 = np.sin(freqs.T).repeat(2, axis=0).astype(np.float64)
    sgn = np.where(np.arange(HD) % 2 == 0, -1.0, 1.0)[:, None]
    cos_t = np.concatenate([c64, c64], axis=0).astype(BF16)           # [128, S]
    sin_t = np.concatenate([s64 * sgn, s64 * sgn], axis=0).astype(BF16)

    # Mask block schedule at [128 k x 512 q] granularity (same for all b, h).
    # Block (qs, kt): full (mask all zero), skip (all <= -30), or masked
    # (multiply exp'd P by exp(mask^T) tile). jlo = first visible 128-q
    # subblock (clean fully-masked prefix only).
    mt_tiles = []  # unique [128, 512] multiplier tiles
    mt_keys = {}
    sched = []  # per qs: list of (kt, mtile_idx | None, jlo)
    for qs in range(QS_TILES):
        lst = []
        for kt in range(KT_TILES):
            blk = mask[qs * 512:(qs + 1) * 512, kt * 128:(kt + 1) * 128]  # [q, k]
            if np.all(blk <= -30.0):
                continue
            jmasked = [np.all(blk[j * 128:(j + 1) * 128] <= -30.0) for j in range(4)]
            jlo = 0
            while jlo < 4 and jmasked[jlo]:
                jlo += 1
            if any(jmasked[jlo:]):
                jlo = 0
            vis = blk[jlo * 128:]
            if np.all(vis == 0.0):
                lst.append((kt, None, jlo))
                continue
            tile_np = np.exp(blk.T.astype(np.float64)).astype(BF16)  # [128k, 512q]
            key = tile_np.tobytes()
            if key not in mt_keys:
                mt_keys[key] = len(mt_tiles)
                mt_tiles.append(tile_np)
            lst.append((kt, mt_keys[key], jlo))
        # every 128-q subblock needs at least one full-width contributing kt
        # so its softmax denominator is well-defined; also the FIRST kt per
        # subblock must be full-width (start=True clears the whole OT bank)
        for j in range(4):
            if not any(e[2] <= j for e in lst):
                lst = [(kt, mi, 0) for (kt, mi, _) in lst]
                break
        if lst and lst[0][2] != 0:
            lst[0] = (lst[0][0], lst[0][1], 0)
        sched.append(lst)
    if not mt_tiles:  # keep the input well-formed even if no masked blocks
        mt_tiles.append(np.ones((128, 512), dtype=BF16))
    mt = np.stack(mt_tiles)  # [U, 128, 512]

    per_core = []
    for c in range(NCORES):
        b, hg = c % 2, c // 2
        per_core.append({
            "xT": np.ascontiguousarray(x[b].T).astype(BF16),            # [D, S]
            "wq": np.ascontiguousarray(
                wq[:, hg * 512:(hg + 1) * 512]).astype(BF16),           # [D, 512]
            "wkv": np.ascontiguousarray(np.concatenate(
                [wk[:, hg * 128:(hg + 1) * 128],
                 wv[:, hg * 128:(hg + 1) * 128]], axis=1)).astype(BF16),  # [D, 256]
            # wo rows permuted to match otn_t layout: pair p holds head 2p+1
            # on partitions 0:64 and head 2p on partitions 64:128
            "wo": np.ascontiguousarray(
                wo[hg * 512:(hg + 1) * 512, :][_WO_PERM, :]).astype(BF16),  # [512, D]
            "cos": cos_t,
            "sin": sin_t,
            "mt": mt,
        })
    return per_core, sched, mt.shape[0]


def _build_program(sched, U):
    import concourse.bass as bass
    import concourse.mybir as mybir
    import concourse.tile as tile
    from concourse import bacc

    dt = mybir.dt
    bf, f32 = dt.bfloat16, dt.float32
    AF = mybir.ActivationFunctionType

    nc = bacc.Bacc("TRN2", target_bir_lowering=False, debug=False,
                   num_devices=NCORES)

    xT = nc.dram_tensor("xT", [D, S], bf, kind="ExternalInput")
    wq = nc.dram_tensor("wq", [D, 512], bf, kind="ExternalInput")
    wkv = nc.dram_tensor("wkv", [D, 256], bf, kind="ExternalInput")
    wo = nc.dram_tensor("wo", [512, D], bf, kind="ExternalInput")
    cos = nc.dram_tensor("cos", [128, S], bf, kind="ExternalInput")
    sin = nc.dram_tensor("sin", [128, S], bf, kind="ExternalInput")
    mt = nc.dram_tensor("mt", [U, 128, 512], bf, kind="ExternalInput")
    out = nc.dram_tensor("out", [S, D], bf, kind="ExternalOutput")

    # pair-swap permutation (block-diag over the two stacked 64-row heads)
    perm_np = np.zeros((128, 128), dtype=BF16)
    for d in range(128):
        perm_np[d ^ 1, d] = 1
    perm_dram = nc.inline_tensor(np.ascontiguousarray(perm_np), name="perm")
    ident_dram = nc.inline_tensor(np.eye(128, dtype=BF16), name="ident")

    with tile.TileContext(nc) as tc:
        with tc.tile_pool(name="const", bufs=1) as cp:
            # DMA issue order matters: the first projection matmuls need
            # x chunk 0 + the weight chunks, so those go first; everything
            # needed later (wo, mask tiles, rope tables) queues after.
            wq_sb = cp.tile([128, DC, 512], bf)
            wkv_sb = cp.tile([128, DC, 256], bf)
            xbig0 = cp.tile([128, DC, 512], bf, name="xbig0")
            wq_r = wq.ap().rearrange("(c p) m -> p c m", p=128)
            wkv_r = wkv.ap().rearrange("(c p) m -> p c m", p=128)
            xT_r = xT.ap().rearrange("(c p) s -> p c s", p=128)
            for dc in range(DC):
                nc.sync.dma_start(xbig0[:, dc, :], xT_r[:, dc, 0:512])
                nc.sync.dma_start(wq_sb[:, dc, :], wq_r[:, dc, :])
                nc.sync.dma_start(wkv_sb[:, dc, :], wkv_r[:, dc, :])
            perm_sb = cp.tile([128, 128], bf)
            nc.sync.dma_start(perm_sb[:], perm_dram.ap())
            ident_sb = cp.tile([128, 128], bf)
            nc.sync.dma_start(ident_sb[:], ident_dram.ap())
            cos_sb = cp.tile([128, S], bf)
            nc.sync.dma_start(cos_sb[:], cos.ap())
            sin_sb = cp.tile([128, S], bf)
            nc.sync.dma_start(sin_sb[:], sin.ap())
            mt_sb = cp.tile([128, U, 512], bf)
            nc.sync.dma_start(mt_sb[:], mt.ap().rearrange("u p q -> p u q"))
            wo_sb = cp.tile([128, NPAIR, D], bf)
            nc.sync.dma_start(wo_sb[:], wo.ap().rearrange("(g p) n -> p g n", p=128))
            qt_sb = cp.tile([128, NPAIR, S], bf)    # [dh-pair, pair, s]
            # K^T zero-padded to a full 128 contraction so score matmuls stay
            # in the default 128x128 PE mode (no tiling-mode drains): slot 0
            # = [K | 0] (even head of the pair), slot 1 = [0 | K] (odd head)
            kt_sb = cp.tile([128, 2, 2, S], bf)     # [dh|0 halves, kv, slot, s]
            nc.vector.memset(kt_sb[64:128, :, 0, :], 0.0)
            nc.vector.memset(kt_sb[0:64, :, 1, :], 0.0)
            # PV stationary operand per (kv, kt): [ones(64) | V(64)] so the
            # softmax denominator lands on PSUM partitions 0:64 (partition 0
            # feeds the gpsimd broadcast) and O^T on partitions 64:128
            vone_sb = cp.tile([128, 2, KT_TILES, 128], bf)  # [k%128, kv, kt, 1|dh]
            nc.vector.memset(vone_sb[:, :, :, 0:HD], 1.0)

            # ---------------- phase 1: projections + rope ----------------
            with (
                tc.tile_pool(name="xt", bufs=3) as xp,
                tc.tile_pool(name="raw", bufs=3) as rawp,
                tc.tile_pool(name="rtmp", bufs=3) as rtp,
                tc.tile_pool(name="ps_pr", bufs=1, space="PSUM") as ppr,
                tc.tile_pool(name="ps_sw", bufs=1, space="PSUM") as psw,
                tc.tile_pool(name="ps_vt", bufs=1, space="PSUM") as pvt,
            ):
                for st in range(S // 512):
                    s0 = st * 512
                    if st == 0:
                        xbig = xbig0
                    else:
                        xbig = xp.tile([128, DC, 512], bf)
                        # split the 2MB chunk load across DMA queues
                        for dc2 in range(0, DC, 2):
                            nc.sync.dma_start(xbig[:, dc2:dc2 + 2, :],
                                              xT_r[:, dc2:dc2 + 2, s0:s0 + 512])
                    qps = [ppr.tile([128, 512], f32, tag=f"q{i}", name=f"qp{i}")
                           for i in range(NPAIR)]
                    kp = ppr.tile([128, 512], f32, tag="kp")
                    vp = ppr.tile([128, 512], f32, tag="vp")
                    for dc in range(DC):
                        st_, sp_ = (dc == 0), (dc == DC - 1)
                        for i in range(NPAIR):
                            nc.tensor.matmul(qps[i][:],
                                             lhsT=wq_sb[:, dc, i * 128:(i + 1) * 128],
                                             rhs=xbig[:, dc, :], start=st_, stop=sp_)
                        nc.tensor.matmul(kp[:], lhsT=wkv_sb[:, dc, 0:128],
                                         rhs=xbig[:, dc, :], start=st_, stop=sp_)
                        nc.tensor.matmul(vp[:], lhsT=wkv_sb[:, dc, 128:256],
                                         rhs=xbig[:, dc, :], start=st_, stop=sp_)
                    # raw copies to SBUF (also the swap-matmul inputs),
                    # split across ScalarE and DVE to halve the chain
                    qrs = []
                    for i in range(NPAIR):
                        qr = rawp.tile([128, 512], bf, tag=f"q{i}r", name=f"q{i}r")
                        if i % 2 == 0:
                            nc.scalar.copy(qr[:], qps[i][:])
                        else:
                            nc.vector.tensor_copy(qr[:], qps[i][:])
                        qrs.append(qr)
                    kr = rawp.tile([128, 512], bf, tag="kr")
                    nc.vector.tensor_copy(kr[:], kp[:])
                    vr = rawp.tile([128, 512], bf, tag="vr")
                    nc.scalar.copy(vr[:], vp[:])
                    # V: transpose VT rows back to natural [k, dh]
                    for kv in range(2):
                        pl = kv * 64
                        for j in range(4):
                            vtp = pvt.tile([128, HD], bf)
                            nc.tensor.transpose(
                                vtp[:], vr[pl:pl + 64, j * 128:(j + 1) * 128],
                                ident_sb[pl:pl + 64, pl:pl + 64])
                            nc.vector.tensor_copy(
                                vone_sb[:, kv, 4 * st + j, HD:128], vtp[:])
                    # rope Q (pairs stacked on 128 partitions)
                    for i, qr in enumerate(qrs):
                        swp = psw.tile([128, 512], f32, tag="sw")
                        nc.tensor.matmul(swp[:], lhsT=perm_sb[:], rhs=qr[:],
                                         start=True, stop=True)
                        t_sin = rtp.tile([128, 512], bf, tag="tsin")
                        nc.vector.tensor_mul(t_sin[:], swp[:], sin_sb[:, s0:s0 + 512])
                        t_cos = rtp.tile([128, 512], bf, tag="tcos")
                        nc.vector.tensor_mul(t_cos[:], qr[:], cos_sb[:, s0:s0 + 512])
                        nc.vector.tensor_add(qt_sb[:, i, s0:s0 + 512],
                                             t_sin[:], t_cos[:])
                    # rope K (both kv heads stacked); write into the dup'd
                    # halves, then DMA-duplicate across partition halves
                    ksw = psw.tile([128, 512], f32, tag="sw")
                    nc.tensor.matmul(ksw[:], lhsT=perm_sb[:], rhs=kr[:],
                                     start=True, stop=True)
                    k_sin = rtp.tile([128, 512], bf, tag="tsin")
                    nc.vector.tensor_mul(k_sin[:], ksw[:], sin_sb[:, s0:s0 + 512])
                    k_cos = rtp.tile([128, 512], bf, tag="tcos")
                    nc.vector.tensor_mul(k_cos[:], kr[:], cos_sb[:, s0:s0 + 512])
                    nc.vector.tensor_add(kt_sb[0:64, 0, 0, s0:s0 + 512],
                                         k_sin[0:64, :], k_cos[0:64, :])
                    nc.vector.tensor_add(kt_sb[64:128, 1, 1, s0:s0 + 512],
                                         k_sin[64:128, :], k_cos[64:128, :])
                    nc.sync.dma_start(kt_sb[64:128, 0, 1, s0:s0 + 512],
                                      kt_sb[0:64, 0, 0, s0:s0 + 512])
                    nc.sync.dma_start(kt_sb[0:64, 1, 0, s0:s0 + 512],
                                      kt_sb[64:128, 1, 1, s0:s0 + 512])

            # ---------------- phase 2: attention + wo ----------------
            with (
                tc.tile_pool(name="pp", bufs=3) as ppool,
                tc.tile_pool(name="rcp", bufs=2) as rcp,
                tc.tile_pool(name="otn", bufs=2) as otp,
                tc.tile_pool(name="wsb", bufs=3) as wsp,
                tc.tile_pool(name="ps_s", bufs=2, space="PSUM") as pss,
                tc.tile_pool(name="ps_o", bufs=1, space="PSUM") as pso,
                tc.tile_pool(name="ps_w", bufs=2, space="PSUM") as psw2,
            ):
                for qs in range(QS_TILES):
                    q0 = qs * 512
                    kts = sched[qs]
                    last_kt = max(e[0] for e in kts)
                    otn_t = otp.tile([128, NPAIR, 512], bf)  # [dh-pair, pair, q]
                    for p in range(NPAIR):
                        kv = p // 2
                        ot0 = pso.tile([128, 512], f32, tag="ot0", name="ot0")
                        ot1 = pso.tile([128, 512], f32, tag="ot1", name="ot1")
                        for kt, mi, jlo in kts:
                            nq = 512 - jlo * 128
                            ql = q0 + jlo * 128
                            spp = pss.tile([128, 2, 512], f32)
                            for h2 in range(2):
                                nc.tensor.matmul(
                                    spp[:, h2, jlo * 128:512],
                                    lhsT=kt_sb[:, kv, h2,
                                               kt * 128:(kt + 1) * 128],
                                    rhs=qt_sb[:, p, ql:q0 + 512],
                                    start=True, stop=True)
                            pt = ppool.tile([128, 2, 512], bf)
                            nc.scalar.activation(pt[:, :, jlo * 128:512],
                                                 spp[:, :, jlo * 128:512], AF.Exp,
                                                 scale=1.0 / np.sqrt(HD))
                            if mi is not None:
                                for h2 in range(2):
                                    nc.vector.tensor_mul(
                                        pt[:, h2, jlo * 128:512],
                                        pt[:, h2, jlo * 128:512],
                                        mt_sb[:, mi, jlo * 128:512])
                            for h2, ot in ((0, ot0), (1, ot1)):
                                nc.tensor.matmul(
                                    ot[:, jlo * 128:512],
                                    lhsT=vone_sb[:, kv, kt, :],
                                    rhs=pt[:, h2, jlo * 128:512],
                                    start=(kt == kts[0][0]),
                                    stop=(kt == last_kt))
                        # evacuate OT to SBUF with one copy (frees the PSUM
                        # bank for the next pair fast), then normalize off-
                        # PSUM: denom partition 0 -> fast recip -> gpsimd
                        # broadcast -> DVE multiply on the O half (64:128).
                        # Even head writes otn_t[64:128] directly; odd head
                        # goes via SBUF->SBUF DMA to otn_t[0:64].
                        for h2, ot in ((0, ot0), (1, ot1)):
                            otu = rcp.tile([128, 512], f32, tag=f"otu{h2}",
                                           name=f"otu{h2}")
                            nc.vector.tensor_copy(otu[:], ot[:])
                            rc = rcp.tile([128, 512], f32, tag="rc")
                            nc.vector.reciprocal_approx_fast(
                                out=rc[0:1, :], in_=otu[0:1, :])
                            rbs = rcp.tile([128, 512], f32, tag="rbs")
                            nc.gpsimd.partition_broadcast(rbs[:], rc[0:1, :])
                            if h2 == 0:
                                nc.vector.tensor_mul(otn_t[64:128, p, :],
                                                     otu[64:128, :],
                                                     rbs[64:128, :])
                            else:
                                otmp = rcp.tile([128, 512], bf, tag="otmp")
                                nc.vector.tensor_mul(otmp[64:128, :],
                                                     otu[64:128, :],
                                                     rbs[64:128, :])
                                nc.gpsimd.dma_start(otn_t[0:64, p, :],
                                                    otmp[64:128, :])
                    # wo for this q-stripe
                    for j in range(4):
                        sq0 = q0 + j * 128
                        for nb in range(4):
                            wp = psw2.tile([128, 512], f32)
                            for p in range(NPAIR):
                                nc.tensor.matmul(
                                    wp[:],
                                    lhsT=otn_t[:, p, j * 128:(j + 1) * 128],
                                    rhs=wo_sb[:, p, nb * 512:(nb + 1) * 512],
                                    start=(p == 0), stop=(p == NPAIR - 1))
                            wsb = wsp.tile([128, 512], bf)
                            if (j + nb) % 2 == 0:
                                nc.vector.tensor_copy(wsb[:], wp[:])
                            else:
                                nc.scalar.copy(wsb[:], wp[:])
                            nc.sync.dma_start(
                                out.ap()[sq0:sq0 + 128, nb * 512:(nb + 1) * 512],
                                wsb[:])
    nc.compile()
    return nc


def kernel(x, wq, wk, wv, wo, freqs, mask, start_pos):
    sys.path.insert(0, "/opt/trn_rl_repo")
    from concourse.bass_utils import run_bass_kernel_spmd

    x = np.asarray(x, dtype=np.float32)
    per_core, sched, U = _host_prepare(
        x, np.asarray(wq, np.float32), np.asarray(wk, np.float32),
        np.asarray(wv, np.float32), np.asarray(wo, np.float32),
        np.asarray(freqs, np.float32), np.asarray(mask, np.float32))

    nc = _build_program(sched, U)

    trace = bool(int(os.environ.get("BASSKERNEL_TRACE", "0")))
    if trace and "antenv.axon_hooks" not in sys.modules:
        # profile-hook shim (the trimmed antenv package lacks axon_hooks)
        try:
            import types

            if "/root/.axon_site" not in sys.path:
                sys.path.insert(0, "/root/.axon_site")
            from trn_agent_boot.trn_boot import _ntff_profile_via_ctypes

            _hook = _ntff_profile_via_ctypes("/opt/axon/libaxon_pjrt.so")
            _mod = types.ModuleType("antenv.axon_hooks")
            _mod.get_axon_ntff_profile_hook = lambda: _hook
            _mod.set_axon_ntff_profile_hook = lambda h: None
            sys.modules["antenv.axon_hooks"] = _mod
        except Exception:
            trace = False
    res = run_bass_kernel_spmd(nc, per_core, core_ids=list(range(NCORES)),
                               trace=trace)
    if trace:
        kernel._last_exec_time_ns = res.exec_time_ns
        kernel._last_profile = res.profile_json
    acc = np.zeros((B, S, D), np.float64)
    for c in range(NCORES):
        acc[c % 2] += res.results[c]["out"].astype(np.float64)
    return acc.astype(np.float32)


# revision 41
# speedup vs baseline: 1.3410x; 1.0182x over previous
"""GQA prefill attention (B=2, S=2048, D=2048, H=32, KV=8, HD=64) on 8 trn2 cores.

Sharding: batch x head-group. Core c owns batch b=c%2 and head-group hg=c//2:
q-heads [8hg, 8hg+8), kv-heads {2hg, 2hg+1}; computes its partial of
out[b] = attn_out @ wo_hg; host sums the 4 partials per batch.

Device kernel (per core, bf16 matmuls / fp32 PSUM):
  QT[dh,s] per head-pair (two 64-row heads stacked on 128 partitions)
  KT[dh,s] per kv head, duplicated onto both partition halves (row tiling)
  V[k,dh] natural layout via PE transpose; 65th column of ones
  RoPE via pair-swap permutation matmul + elementwise cos/sin tables
  ST[k,q] = KT^T @ QT for a head pair: two concurrent row-tiled K=64 matmuls
  P = exp(ST/8) one ScalarE instr per (pair, kt) over both PSUM banks
  OT[dh|sum, q] += [V | 1]^T-chunks @ P  (N=512 streaming PV, no LDW churn)
  normalize: recip(denom row) -> PE ones-broadcast -> DVE multiply
  out_partial[s,:] = OTn-chunks^T @ wo_hg   (bf16 partial to HBM)
"""

import os
import sys

import numpy as np
import ml_dtypes

BF16 = ml_dtypes.bfloat16

B, S, D, H, KV, HD = 2, 2048, 2048, 32, 8, 64
NCORES = 8
HPC = 8            # q-heads per core
NPAIR = 4          # head pairs per core
QS_TILES = S // 512
KT_TILES = S // 128
DC = D // 128      # contraction chunks for the projections

# per-pair wo row order: odd head first (otn_t partitions 0:64), even second
_WO_PERM = np.concatenate(
    [np.concatenate([np.arange((2 * p + 1) * 64, (2 * p + 2) * 64),
                     np.arange(2 * p * 64, (2 * p + 1) * 64)])
     for p in range(4)])


def _host_prepare(x, wq, wk, wv, wo, freqs, mask):
    """Build per-core device inputs + the mask block schedule."""
    # RoPE tables in the [dh-on-partitions, s] layout used by QT/KT.
    # Two 64-row head copies stacked (head pairs live on 128 partitions).
    # rope: out[2j]   = t[2j] cos - t[2j+1] sin
    #       out[2j+1] = t[2j] sin + t[2j+1] cos
    # with swap(t)[d] = t[d^1]:  out[d] = t[d]*cos[d] + swap(t)[d]*sgn(d)*sin[d]
    c64 = np.cos(freqs.T).repeat(2, axis=0).astype(np.float64)  # [64, S]
    s64 = np.sin(freqs.T).repeat(2, axis=0).astype(np.float64)
    sgn = np.where(np.arange(HD) % 2 == 0, -1.0, 1.0)[:, None]
    cos_t = np.concatenate([c64, c64], axis=0).astype(BF16)           # [128, S]
    sin_t = np.concatenate([s64 * sgn, s64 * sgn], axis=0).astype(BF16)

    # Mask block schedule at [128 k x 512 q] granularity (same for all b, h).
    # Block (qs, kt): full (mask all zero), skip (all <= -30), or masked
    # (multiply exp'd P by exp(mask^T) tile). jlo = first visible 128-q
    # subblock (clean fully-masked prefix only).
    mt_tiles = []  # unique [128, 512] multiplier tiles
    mt_keys = {}
    sched = []  # per qs: list of (kt, mtile_idx | None, jlo)
    for qs in range(QS_TILES):
        lst = []
        for kt in range(KT_TILES):
            blk = mask[qs * 512:(qs + 1) * 512, kt * 128:(kt + 1) * 128]  # [q, k]
            if np.all(blk <= -30.0):
                continue
            jmasked = [np.all(blk[j * 128:(j + 1) * 128] <= -30.0) for j in range(4)]
            jlo = 0
            while jlo < 4 and jmasked[jlo]:
                jlo += 1
            if any(jmasked[jlo:]):
                jlo = 0
            vis = blk[jlo * 128:]
            if np.all(vis == 0.0):
                lst.append((kt, None, jlo))
                continue
            tile_np = np.exp(blk.T.astype(np.float64)).astype(BF16)  # [128k, 512q]
            key = tile_np.tobytes()
            if key not in mt_keys:
                mt_keys[key] = len(mt_tiles)
                mt_tiles.append(tile_np)
            lst.append((kt, mt_keys[key], jlo))
        # every 128-q subblock needs at least one full-width contributing kt
        # so its softmax denominator is well-defined; also the FIRST kt per
        # subblock must be full-width (start=True clears the whole OT bank)
        for j in range(4):
            if not any(e[2] <= j for e in lst):
                lst = [(kt, mi, 0) for (kt, mi, _) in lst]
                break
        if lst and lst[0][2] != 0:
            lst[0] = (lst[0][0], lst[0][1], 0)
        sched.append(lst)
    if not mt_tiles:  # keep the input well-formed even if no masked blocks
        mt_tiles.append(np.ones((128, 512), dtype=BF16))
    mt = np.stack(mt_tiles)  # [U, 128, 512]

    per_core = []
    for c in range(NCORES):
        b, hg = c % 2, c // 2
        per_core.append({
            "xT": np.ascontiguousarray(x[b].T).astype(BF16),            # [D, S]
            "wq": np.ascontiguousarray(
                wq[:, hg * 512:(hg + 1) * 512]).astype(BF16),           # [D, 512]
            "wkv": np.ascontiguousarray(np.concatenate(
                [wk[:, hg * 128:(hg + 1) * 128],
                 wv[:, hg * 128:(hg + 1) * 128]], axis=1)).astype(BF16),  # [D, 256]
            # wo rows permuted to match otn_t layout: pair p holds head 2p+1
            # on partitions 0:64 and head 2p on partitions 64:128
            "wo": np.ascontiguousarray(
                wo[hg * 512:(hg + 1) * 512, :][_WO_PERM, :]).astype(BF16),  # [512, D]
            "cos": cos_t,
            "sin": sin_t,
            "mt": mt,
        })
    return per_core, sched, mt.shape[0]


def _build_program(sched, U):
    import concourse.bass as bass
    import concourse.mybir as mybir
    import concourse.tile as tile
    from concourse import bacc

    dt = mybir.dt
    bf, f32 = dt.bfloat16, dt.float32
    AF = mybir.ActivationFunctionType

    nc = bacc.Bacc("TRN2", target_bir_lowering=False, debug=False,
                   num_devices=NCORES)

    xT = nc.dram_tensor("xT", [D, S], bf, kind="ExternalInput")
    wq = nc.dram_tensor("wq", [D, 512], bf, kind="ExternalInput")
    wkv = nc.dram_tensor("wkv", [D, 256], bf, kind="ExternalInput")
    wo = nc.dram_tensor("wo", [512, D], bf, kind="ExternalInput")
    cos = nc.dram_tensor("cos", [128, S], bf, kind="ExternalInput")
    sin = nc.dram_tensor("sin", [128, S], bf, kind="ExternalInput")
    mt = nc.dram_tensor("mt", [U, 128, 512], bf, kind="ExternalInput")
    out = nc.dram_tensor("out", [S, D], bf, kind="ExternalOutput")

    # pair-swap permutation (block-diag over the two stacked 64-row heads)
    perm_np = np.zeros((128, 128), dtype=BF16)
    for d in range(128):
        perm_np[d ^ 1, d] = 1
    perm_dram = nc.inline_tensor(np.ascontiguousarray(perm_np), name="perm")
    ident_dram = nc.inline_tensor(np.eye(128, dtype=BF16), name="ident")

    with tile.TileContext(nc) as tc:
        with tc.tile_pool(name="const", bufs=1) as cp:
            # DMA issue order matters: the first projection matmuls need
            # x chunk 0 + the weight chunks, so those go first; everything
            # needed later (wo, mask tiles, rope tables) queues after.
            wq_sb = cp.tile([128, DC, 512], bf)
            wkv_sb = cp.tile([128, DC, 256], bf)
            xbig0 = cp.tile([128, DC, 512], bf, name="xbig0")
            wq_r = wq.ap().rearrange("(c p) m -> p c m", p=128)
            wkv_r = wkv.ap().rearrange("(c p) m -> p c m", p=128)
            xT_r = xT.ap().rearrange("(c p) s -> p c s", p=128)
            for dc in range(DC):
                nc.sync.dma_start(xbig0[:, dc, :], xT_r[:, dc, 0:512])
                nc.sync.dma_start(wq_sb[:, dc, :], wq_r[:, dc, :])
                nc.sync.dma_start(wkv_sb[:, dc, :], wkv_r[:, dc, :])
            perm_sb = cp.tile([128, 128], bf)
            nc.sync.dma_start(perm_sb[:], perm_dram.ap())
            ident_sb = cp.tile([128, 128], bf)
            nc.sync.dma_start(ident_sb[:], ident_dram.ap())
            cos_sb = cp.tile([128, S], bf)
            nc.sync.dma_start(cos_sb[:], cos.ap())
            sin_sb = cp.tile([128, S], bf)
            nc.sync.dma_start(sin_sb[:], sin.ap())
            mt_sb = cp.tile([128, U, 512], bf)
            nc.sync.dma_start(mt_sb[:], mt.ap().rearrange("u p q -> p u q"))
            wo_sb = cp.tile([128, NPAIR, D], bf)
            nc.sync.dma_start(wo_sb[:], wo.ap().rearrange("(g p) n -> p g n", p=128))
            qt_sb = cp.tile([128, NPAIR, S], bf)    # [dh-pair, pair, s]
            # K^T zero-padded to a full 128 contraction so score matmuls stay
            # in the default 128x128 PE mode (no tiling-mode drains): slot 0
            # = [K | 0] (even head of the pair), slot 1 = [0 | K] (odd head)
            kt_sb = cp.tile([128, 2, 2, S], bf)     # [dh|0 halves, kv, slot, s]
            nc.vector.memset(kt_sb[64:128, :, 0, :], 0.0)
            nc.vector.memset(kt_sb[0:64, :, 1, :], 0.0)
            # PV stationary operand per (kv, kt): [ones(64) | V(64)] so the
            # softmax denominator lands on PSUM partitions 0:64 (partition 0
            # feeds the gpsimd broadcast) and O^T on partitions 64:128
            vone_sb = cp.tile([128, 2, KT_TILES, 128], bf)  # [k%128, kv, kt, 1|dh]
            nc.vector.memset(vone_sb[:, :, :, 0:HD], 1.0)

            # ---------------- phase 1: projections + rope ----------------
            with (
                tc.tile_pool(name="xt", bufs=3) as xp,
                tc.tile_pool(name="raw", bufs=3) as rawp,
                tc.tile_pool(name="rtmp", bufs=3) as rtp,
                tc.tile_pool(name="ps_pr", bufs=1, space="PSUM") as ppr,
                tc.tile_pool(name="ps_sw", bufs=1, space="PSUM") as psw,
                tc.tile_pool(name="ps_vt", bufs=1, space="PSUM") as pvt,
            ):
                for st in range(S // 512):
                    s0 = st * 512
                    if st == 0:
                        xbig = xbig0
                    else:
                        xbig = xp.tile([128, DC, 512], bf)
                        # split the 2MB chunk load across DMA queues
                        for dc2 in range(0, DC, 2):
                            nc.sync.dma_start(xbig[:, dc2:dc2 + 2, :],
                                              xT_r[:, dc2:dc2 + 2, s0:s0 + 512])
                    qps = [ppr.tile([128, 512], f32, tag=f"q{i}", name=f"qp{i}")
                           for i in range(NPAIR)]
                    kp = ppr.tile([128, 512], f32, tag="kp")
                    vp = ppr.tile([128, 512], f32, tag="vp")
                    for dc in range(DC):
                        st_, sp_ = (dc == 0), (dc == DC - 1)
                        for i in range(NPAIR):
                            nc.tensor.matmul(qps[i][:],
                                             lhsT=wq_sb[:, dc, i * 128:(i + 1) * 128],
                                             rhs=xbig[:, dc, :], start=st_, stop=sp_)
                        nc.tensor.matmul(kp[:], lhsT=wkv_sb[:, dc, 0:128],
                                         rhs=xbig[:, dc, :], start=st_, stop=sp_)
                        nc.tensor.matmul(vp[:], lhsT=wkv_sb[:, dc, 128:256],
                                         rhs=xbig[:, dc, :], start=st_, stop=sp_)
                    # raw copies to SBUF (also the swap-matmul inputs),
                    # split across ScalarE and DVE to halve the chain
                    qrs = []
                    for i in range(NPAIR):
                        qr = rawp.tile([128, 512], bf, tag=f"q{i}r", name=f"q{i}r")
                        if i % 2 == 0:
                            nc.scalar.copy(qr[:], qps[i][:])
                        else:
                            nc.vector.tensor_copy(qr[:], qps[i][:])
                        qrs.append(qr)
                    kr = rawp.tile([128, 512], bf, tag="kr")
                    nc.vector.tensor_copy(kr[:], kp[:])
                    vr = rawp.tile([128, 512], bf, tag="vr")
                    nc.scalar.copy(vr[:], vp[:])
                    # V: transpose VT rows back to natural [k, dh]
                    for kv in range(2):
                        pl = kv * 64
                        for j in range(4):
                            vtp = pvt.tile([128, HD], bf)
                            nc.tensor.transpose(
                                vtp[:], vr[pl:pl + 64, j * 128:(j + 1) * 128],
                                ident_sb[pl:pl + 64, pl:pl + 64])
                            nc.vector.tensor_copy(
                                vone_sb[:, kv, 4 * st + j, HD:128], vtp[:])
                    # rope Q (pairs stacked on 128 partitions)
                    for i, qr in enumerate(qrs):
                        swp = psw.tile([128, 512], f32, tag="sw")
                        nc.tensor.matmul(swp[:], lhsT=perm_sb[:], rhs=qr[:],
                                         start=True, stop=True)
                        t_sin = rtp.tile([128, 512], bf, tag="tsin")
                        nc.vector.tensor_mul(t_sin[:], swp[:], sin_sb[:, s0:s0 + 512])
                        t_cos = rtp.tile([128, 512], bf, tag="tcos")
                        nc.vector.tensor_mul(t_cos[:], qr[:], cos_sb[:, s0:s0 + 512])
                        nc.vector.tensor_add(qt_sb[:, i, s0:s0 + 512],
                                             t_sin[:], t_cos[:])
                    # rope K (both kv heads stacked); write into the dup'd
                    # halves, then DMA-duplicate across partition halves
                    ksw = psw.tile([128, 512], f32, tag="sw")
                    nc.tensor.matmul(ksw[:], lhsT=perm_sb[:], rhs=kr[:],
                                     start=True, stop=True)
                    k_sin = rtp.tile([128, 512], bf, tag="tsin")
                    nc.vector.tensor_mul(k_sin[:], ksw[:], sin_sb[:, s0:s0 + 512])
                    k_cos = rtp.tile([128, 512], bf, tag="tcos")
                    nc.vector.tensor_mul(k_cos[:], kr[:], cos_sb[:, s0:s0 + 512])
                    nc.vector.tensor_add(kt_sb[0:64, 0, 0, s0:s0 + 512],
                                         k_sin[0:64, :], k_cos[0:64, :])
                    nc.vector.tensor_add(kt_sb[64:128, 1, 1, s0:s0 + 512],
                                         k_sin[64:128, :], k_cos[64:128, :])
                    nc.sync.dma_start(kt_sb[64:128, 0, 1, s0:s0 + 512],
                                      kt_sb[0:64, 0, 0, s0:s0 + 512])
                    nc.sync.dma_start(kt_sb[0:64, 1, 0, s0:s0 + 512],
                                      kt_sb[64:128, 1, 1, s0:s0 + 512])

            # ---------------- phase 2: attention + wo ----------------
            with (
                tc.tile_pool(name="pp", bufs=4) as ppool,
                tc.tile_pool(name="rcp", bufs=3) as rcp,
                tc.tile_pool(name="otn", bufs=2) as otp,
                tc.tile_pool(name="wsb", bufs=4) as wsp,
                tc.tile_pool(name="ps_s", bufs=2, space="PSUM") as pss,
                tc.tile_pool(name="ps_o", bufs=1, space="PSUM") as pso,
                tc.tile_pool(name="ps_w", bufs=2, space="PSUM") as psw2,
            ):
                for qs in range(QS_TILES):
                    q0 = qs * 512
                    kts = sched[qs]
                    last_kt = max(e[0] for e in kts)
                    otn_t = otp.tile([128, NPAIR, 512], bf)  # [dh-pair, pair, q]
                    for p in range(NPAIR):
                        kv = p // 2
                        ot0 = pso.tile([128, 512], f32, tag="ot0", name="ot0")
                        ot1 = pso.tile([128, 512], f32, tag="ot1", name="ot1")
                        for kt, mi, jlo in kts:
                            nq = 512 - jlo * 128
                            ql = q0 + jlo * 128
                            spp = pss.tile([128, 2, 512], f32)
                            for h2 in range(2):
                                nc.tensor.matmul(
                                    spp[:, h2, jlo * 128:512],
                                    lhsT=kt_sb[:, kv, h2,
                                               kt * 128:(kt + 1) * 128],
                                    rhs=qt_sb[:, p, ql:q0 + 512],
                                    start=True, stop=True)
                            pt = ppool.tile([128, 2, 512], bf)
                            nc.scalar.activation(pt[:, :, jlo * 128:512],
                                                 spp[:, :, jlo * 128:512], AF.Exp,
                                                 scale=1.0 / np.sqrt(HD))
                            if mi is not None:
                                for h2 in range(2):
                                    nc.vector.tensor_mul(
                                        pt[:, h2, jlo * 128:512],
                                        pt[:, h2, jlo * 128:512],
                                        mt_sb[:, mi, jlo * 128:512])
                            for h2, ot in ((0, ot0), (1, ot1)):
                                nc.tensor.matmul(
                                    ot[:, jlo * 128:512],
                                    lhsT=vone_sb[:, kv, kt, :],
                                    rhs=pt[:, h2, jlo * 128:512],
                                    start=(kt == kts[0][0]),
                                    stop=(kt == last_kt))
                        # evacuate OT to SBUF with one copy (frees the PSUM
                        # bank for the next pair fast), then normalize off-
                        # PSUM: denom partition 0 -> fast recip -> gpsimd
                        # broadcast -> DVE multiply on the O half (64:128).
                        # Even head writes otn_t[64:128] directly; odd head
                        # goes via SBUF->SBUF DMA to otn_t[0:64].
                        for h2, ot in ((0, ot0), (1, ot1)):
                            otu = rcp.tile([128, 512], f32, tag=f"otu{h2}",
                                           name=f"otu{h2}")
                            nc.vector.tensor_copy(otu[:], ot[:])
                            rc = rcp.tile([128, 512], f32, tag="rc")
                            nc.vector.reciprocal_approx_fast(
                                out=rc[0:1, :], in_=otu[0:1, :])
                            rbs = rcp.tile([128, 512], f32, tag="rbs")
                            nc.gpsimd.partition_broadcast(rbs[:], rc[0:1, :])
                            if h2 == 0:
                                nc.vector.tensor_mul(otn_t[64:128, p, :],
                                                     otu[64:128, :],
                                                     rbs[64:128, :])
                            else:
                                otmp = rcp.tile([128, 512], bf, tag="otmp")
                                nc.vector.tensor_mul(otmp[64:128, :],
                                                     otu[64:128, :],
                                                     rbs[64:128, :])
                                nc.gpsimd.dma_start(otn_t[0:64, p, :],
                                                    otmp[64:128, :])
                    # wo for this q-stripe
                    for j in range(4):
                        sq0 = q0 + j * 128
                        for nb in range(4):
                            wp = psw2.tile([128, 512], f32)
                            for p in range(NPAIR):
                                nc.tensor.matmul(
                                    wp[:],
                                    lhsT=otn_t[:, p, j * 128:(j + 1) * 128],
                                    rhs=wo_sb[:, p, nb * 512:(nb + 1) * 512],
                                    start=(p == 0), stop=(p == NPAIR - 1))
                            wsb = wsp.tile([128, 512], bf)
                            if (j + nb) % 2 == 0:
                                nc.vector.tensor_copy(wsb[:], wp[:])
                            else:
                                nc.scalar.copy(wsb[:], wp[:])
                            nc.sync.dma_start(
                                out.ap()[sq0:sq0 + 128, nb * 512:(nb + 1) * 512],
                                wsb[:])
    nc.compile()
    return nc


def kernel(x, wq, wk, wv, wo, freqs, mask, start_pos):
    sys.path.insert(0, "/opt/trn_rl_repo")
    from concourse.bass_utils import run_bass_kernel_spmd

    x = np.asarray(x, dtype=np.float32)
    per_core, sched, U = _host_prepare(
        x, np.asarray(wq, np.float32), np.asarray(wk, np.float32),
        np.asarray(wv, np.float32), np.asarray(wo, np.float32),
        np.asarray(freqs, np.float32), np.asarray(mask, np.float32))

    nc = _build_program(sched, U)

    trace = bool(int(os.environ.get("BASSKERNEL_TRACE", "0")))
    if trace and "antenv.axon_hooks" not in sys.modules:
        # profile-hook shim (the trimmed antenv package lacks axon_hooks)
        try:
            import types

            if "/root/.axon_site" not in sys.path:
                sys.path.insert(0, "/root/.axon_site")
            from trn_agent_boot.trn_boot import _ntff_profile_via_ctypes

            _hook = _ntff_profile_via_ctypes("/opt/axon/libaxon_pjrt.so")
            _mod = types.ModuleType("antenv.axon_hooks")
            _mod.get_axon_ntff_profile_hook = lambda: _hook
            _mod.set_axon_ntff_profile_hook = lambda h: None
            sys.modules["antenv.axon_hooks"] = _mod
        except Exception:
            trace = False
    res = run_bass_kernel_spmd(nc, per_core, core_ids=list(range(NCORES)),
                               trace=trace)
    if trace:
        kernel._last_exec_time_ns = res.exec_time_ns
        kernel._last_profile = res.profile_json
    acc = np.zeros((B, S, D), np.float64)
    for c in range(NCORES):
        acc[c % 2] += res.results[c]["out"].astype(np.float64)
    return acc.astype(np.float32)


# revision 42
# speedup vs baseline: 1.3439x; 1.0022x over previous
"""GQA prefill attention (B=2, S=2048, D=2048, H=32, KV=8, HD=64) on 8 trn2 cores.

Sharding: batch x head-group. Core c owns batch b=c%2 and head-group hg=c//2:
q-heads [8hg, 8hg+8), kv-heads {2hg, 2hg+1}; computes its partial of
out[b] = attn_out @ wo_hg; host sums the 4 partials per batch.

Device kernel (per core, bf16 matmuls / fp32 PSUM):
  QT[dh,s] per head-pair (two 64-row heads stacked on 128 partitions)
  KT[dh,s] per kv head, duplicated onto both partition halves (row tiling)
  V[k,dh] natural layout via PE transpose; 65th column of ones
  RoPE via pair-swap permutation matmul + elementwise cos/sin tables
  ST[k,q] = KT^T @ QT for a head pair: two concurrent row-tiled K=64 matmuls
  P = exp(ST/8) one ScalarE instr per (pair, kt) over both PSUM banks
  OT[dh|sum, q] += [V | 1]^T-chunks @ P  (N=512 streaming PV, no LDW churn)
  normalize: recip(denom row) -> PE ones-broadcast -> DVE multiply
  out_partial[s,:] = OTn-chunks^T @ wo_hg   (bf16 partial to HBM)
"""

import os
import sys

import numpy as np
import ml_dtypes

BF16 = ml_dtypes.bfloat16

B, S, D, H, KV, HD = 2, 2048, 2048, 32, 8, 64
NCORES = 8
HPC = 8            # q-heads per core
NPAIR = 4          # head pairs per core
QS_TILES = S // 512
KT_TILES = S // 128
DC = D // 128      # contraction chunks for the projections

# per-pair wo row order: odd head first (otn_t partitions 0:64), even second
_WO_PERM = np.concatenate(
    [np.concatenate([np.arange((2 * p + 1) * 64, (2 * p + 2) * 64),
                     np.arange(2 * p * 64, (2 * p + 1) * 64)])
     for p in range(4)])


def _host_prepare(x, wq, wk, wv, wo, freqs, mask):
    """Build per-core device inputs + the mask block schedule."""
    # RoPE tables in the [dh-on-partitions, s] layout used by QT/KT.
    # Two 64-row head copies stacked (head pairs live on 128 partitions).
    # rope: out[2j]   = t[2j] cos - t[2j+1] sin
    #       out[2j+1] = t[2j] sin + t[2j+1] cos
    # with swap(t)[d] = t[d^1]:  out[d] = t[d]*cos[d] + swap(t)[d]*sgn(d)*sin[d]
    c64 = np.cos(freqs.T).repeat(2, axis=0).astype(np.float64)  # [64, S]
    s64 = np.sin(freqs.T).repeat(2, axis=0).astype(np.float64)
    sgn = np.where(np.arange(HD) % 2 == 0, -1.0, 1.0)[:, None]
    cos_t = np.concatenate([c64, c64], axis=0).astype(BF16)           # [128, S]
    sin_t = np.concatenate([s64 * sgn, s64 * sgn], axis=0).astype(BF16)

    # Mask block schedule at [128 k x 512 q] granularity (same for all b, h).
    # Block (qs, kt): full (mask all zero), skip (all <= -30), or masked
    # (multiply exp'd P by exp(mask^T) tile). jlo = first visible 128-q
    # subblock (clean fully-masked prefix only).
    mt_tiles = []  # unique [128, 512] multiplier tiles
    mt_keys = {}
    sched = []  # per qs: list of (kt, mtile_idx | None, jlo)
    for qs in range(QS_TILES):
        lst = []
        for kt in range(KT_TILES):
            blk = mask[qs * 512:(qs + 1) * 512, kt * 128:(kt + 1) * 128]  # [q, k]
            if np.all(blk <= -30.0):
                continue
            jmasked = [np.all(blk[j * 128:(j + 1) * 128] <= -30.0) for j in range(4)]
            jlo = 0
            while jlo < 4 and jmasked[jlo]:
                jlo += 1
            if any(jmasked[jlo:]):
                jlo = 0
            vis = blk[jlo * 128:]
            if np.all(vis == 0.0):
                lst.append((kt, None, jlo))
                continue
            tile_np = np.exp(blk.T.astype(np.float64)).astype(BF16)  # [128k, 512q]
            key = tile_np.tobytes()
            if key not in mt_keys:
                mt_keys[key] = len(mt_tiles)
                mt_tiles.append(tile_np)
            lst.append((kt, mt_keys[key], jlo))
        # every 128-q subblock needs at least one full-width contributing kt
        # so its softmax denominator is well-defined; also the FIRST kt per
        # subblock must be full-width (start=True clears the whole OT bank)
        for j in range(4):
            if not any(e[2] <= j for e in lst):
                lst = [(kt, mi, 0) for (kt, mi, _) in lst]
                break
        if lst and lst[0][2] != 0:
            lst[0] = (lst[0][0], lst[0][1], 0)
        sched.append(lst)
    if not mt_tiles:  # keep the input well-formed even if no masked blocks
        mt_tiles.append(np.ones((128, 512), dtype=BF16))
    mt = np.stack(mt_tiles)  # [U, 128, 512]

    per_core = []
    for c in range(NCORES):
        b, hg = c % 2, c // 2
        per_core.append({
            "xT": np.ascontiguousarray(x[b].T).astype(BF16),            # [D, S]
            "wq": np.ascontiguousarray(
                wq[:, hg * 512:(hg + 1) * 512]).astype(BF16),           # [D, 512]
            "wkv": np.ascontiguousarray(np.concatenate(
                [wk[:, hg * 128:(hg + 1) * 128],
                 wv[:, hg * 128:(hg + 1) * 128]], axis=1)).astype(BF16),  # [D, 256]
            # wo rows permuted to match otn_t layout: pair p holds head 2p+1
            # on partitions 0:64 and head 2p on partitions 64:128
            "wo": np.ascontiguousarray(
                wo[hg * 512:(hg + 1) * 512, :][_WO_PERM, :]).astype(BF16),  # [512, D]
            "cos": cos_t,
            "sin": sin_t,
            "mt": mt,
        })
    return per_core, sched, mt.shape[0]


def _build_program(sched, U):
    import concourse.bass as bass
    import concourse.mybir as mybir
    import concourse.tile as tile
    from concourse import bacc

    dt = mybir.dt
    bf, f32 = dt.bfloat16, dt.float32
    AF = mybir.ActivationFunctionType

    nc = bacc.Bacc("TRN2", target_bir_lowering=False, debug=False,
                   num_devices=NCORES)

    xT = nc.dram_tensor("xT", [D, S], bf, kind="ExternalInput")
    wq = nc.dram_tensor("wq", [D, 512], bf, kind="ExternalInput")
    wkv = nc.dram_tensor("wkv", [D, 256], bf, kind="ExternalInput")
    wo = nc.dram_tensor("wo", [512, D], bf, kind="ExternalInput")
    cos = nc.dram_tensor("cos", [128, S], bf, kind="ExternalInput")
    sin = nc.dram_tensor("sin", [128, S], bf, kind="ExternalInput")
    mt = nc.dram_tensor("mt", [U, 128, 512], bf, kind="ExternalInput")
    out = nc.dram_tensor("out", [S, D], bf, kind="ExternalOutput")

    # pair-swap permutation (block-diag over the two stacked 64-row heads)
    perm_np = np.zeros((128, 128), dtype=BF16)
    for d in range(128):
        perm_np[d ^ 1, d] = 1
    perm_dram = nc.inline_tensor(np.ascontiguousarray(perm_np), name="perm")
    ident_dram = nc.inline_tensor(np.eye(128, dtype=BF16), name="ident")

    with tile.TileContext(nc) as tc:
        with tc.tile_pool(name="const", bufs=1) as cp:
            # DMA issue order matters: the first projection matmuls need
            # x chunk 0 + the weight chunks, so those go first; everything
            # needed later (wo, mask tiles, rope tables) queues after.
            wq_sb = cp.tile([128, DC, 512], bf)
            wkv_sb = cp.tile([128, DC, 256], bf)
            xbig0 = cp.tile([128, DC, 512], bf, name="xbig0")
            wq_r = wq.ap().rearrange("(c p) m -> p c m", p=128)
            wkv_r = wkv.ap().rearrange("(c p) m -> p c m", p=128)
            xT_r = xT.ap().rearrange("(c p) s -> p c s", p=128)
            for dc in range(DC):
                nc.sync.dma_start(xbig0[:, dc, :], xT_r[:, dc, 0:512])
                nc.sync.dma_start(wq_sb[:, dc, :], wq_r[:, dc, :])
                nc.sync.dma_start(wkv_sb[:, dc, :], wkv_r[:, dc, :])
            perm_sb = cp.tile([128, 128], bf)
            nc.sync.dma_start(perm_sb[:], perm_dram.ap())
            ident_sb = cp.tile([128, 128], bf)
            nc.sync.dma_start(ident_sb[:], ident_dram.ap())
            cos_sb = cp.tile([128, S], bf)
            nc.sync.dma_start(cos_sb[:], cos.ap())
            sin_sb = cp.tile([128, S], bf)
            nc.sync.dma_start(sin_sb[:], sin.ap())
            mt_sb = cp.tile([128, U, 512], bf)
            nc.sync.dma_start(mt_sb[:], mt.ap().rearrange("u p q -> p u q"))
            wo_sb = cp.tile([128, NPAIR, D], bf)
            nc.sync.dma_start(wo_sb[:], wo.ap().rearrange("(g p) n -> p g n", p=128))
            qt_sb = cp.tile([128, NPAIR, S], bf)    # [dh-pair, pair, s]
            # K^T zero-padded to a full 128 contraction so score matmuls stay
            # in the default 128x128 PE mode (no tiling-mode drains): slot 0
            # = [K | 0] (even head of the pair), slot 1 = [0 | K] (odd head)
            kt_sb = cp.tile([128, 2, 2, S], bf)     # [dh|0 halves, kv, slot, s]
            nc.vector.memset(kt_sb[64:128, :, 0, :], 0.0)
            nc.vector.memset(kt_sb[0:64, :, 1, :], 0.0)
            # PV stationary operand per (kv, kt): [ones(64) | V(64)] so the
            # softmax denominator lands on PSUM partitions 0:64 (partition 0
            # feeds the gpsimd broadcast) and O^T on partitions 64:128
            vone_sb = cp.tile([128, 2, KT_TILES, 128], bf)  # [k%128, kv, kt, 1|dh]
            nc.vector.memset(vone_sb[:, :, :, 0:HD], 1.0)

            # ---------------- phase 1: projections + rope ----------------
            with (
                tc.tile_pool(name="xt", bufs=3) as xp,
                tc.tile_pool(name="raw", bufs=3) as rawp,
                tc.tile_pool(name="rtmp", bufs=3) as rtp,
                tc.tile_pool(name="ps_pr", bufs=1, space="PSUM") as ppr,
                tc.tile_pool(name="ps_sw", bufs=1, space="PSUM") as psw,
                tc.tile_pool(name="ps_vt", bufs=1, space="PSUM") as pvt,
            ):
                for st in range(S // 512):
                    s0 = st * 512
                    if st == 0:
                        xbig = xbig0
                    else:
                        xbig = xp.tile([128, DC, 512], bf)
                        # split the 2MB chunk load across DMA queues
                        for dc2 in range(0, DC, 2):
                            nc.sync.dma_start(xbig[:, dc2:dc2 + 2, :],
                                              xT_r[:, dc2:dc2 + 2, s0:s0 + 512])
                    qps = [ppr.tile([128, 512], f32, tag=f"q{i}", name=f"qp{i}")
                           for i in range(NPAIR)]
                    kp = ppr.tile([128, 512], f32, tag="kp")
                    vp = ppr.tile([128, 512], f32, tag="vp")
                    for dc in range(DC):
                        st_, sp_ = (dc == 0), (dc == DC - 1)
                        for i in range(NPAIR):
                            nc.tensor.matmul(qps[i][:],
                                             lhsT=wq_sb[:, dc, i * 128:(i + 1) * 128],
                                             rhs=xbig[:, dc, :], start=st_, stop=sp_)
                        nc.tensor.matmul(kp[:], lhsT=wkv_sb[:, dc, 0:128],
                                         rhs=xbig[:, dc, :], start=st_, stop=sp_)
                        nc.tensor.matmul(vp[:], lhsT=wkv_sb[:, dc, 128:256],
                                         rhs=xbig[:, dc, :], start=st_, stop=sp_)
                    # raw copies to SBUF (also the swap-matmul inputs),
                    # split across ScalarE and DVE to halve the chain
                    qrs = []
                    for i in range(NPAIR):
                        qr = rawp.tile([128, 512], bf, tag=f"q{i}r", name=f"q{i}r")
                        if i % 2 == 0:
                            nc.scalar.copy(qr[:], qps[i][:])
                        else:
                            nc.vector.tensor_copy(qr[:], qps[i][:])
                        qrs.append(qr)
                    kr = rawp.tile([128, 512], bf, tag="kr")
                    nc.vector.tensor_copy(kr[:], kp[:])
                    vr = rawp.tile([128, 512], bf, tag="vr")
                    nc.scalar.copy(vr[:], vp[:])
                    # V: transpose VT rows back to natural [k, dh]
                    for kv in range(2):
                        pl = kv * 64
                        for j in range(4):
                            vtp = pvt.tile([128, HD], bf)
                            nc.tensor.transpose(
                                vtp[:], vr[pl:pl + 64, j * 128:(j + 1) * 128],
                                ident_sb[pl:pl + 64, pl:pl + 64])
                            nc.vector.tensor_copy(
                                vone_sb[:, kv, 4 * st + j, HD:128], vtp[:])
                    # rope Q (pairs stacked on 128 partitions)
                    for i, qr in enumerate(qrs):
                        swp = psw.tile([128, 512], f32, tag="sw")
                        nc.tensor.matmul(swp[:], lhsT=perm_sb[:], rhs=qr[:],
                                         start=True, stop=True)
                        t_sin = rtp.tile([128, 512], bf, tag="tsin")
                        nc.vector.tensor_mul(t_sin[:], swp[:], sin_sb[:, s0:s0 + 512])
                        t_cos = rtp.tile([128, 512], bf, tag="tcos")
                        nc.vector.tensor_mul(t_cos[:], qr[:], cos_sb[:, s0:s0 + 512])
                        nc.vector.tensor_add(qt_sb[:, i, s0:s0 + 512],
                                             t_sin[:], t_cos[:])
                    # rope K (both kv heads stacked); write into the dup'd
                    # halves, then DMA-duplicate across partition halves
                    ksw = psw.tile([128, 512], f32, tag="sw")
                    nc.tensor.matmul(ksw[:], lhsT=perm_sb[:], rhs=kr[:],
                                     start=True, stop=True)
                    k_sin = rtp.tile([128, 512], bf, tag="tsin")
                    nc.vector.tensor_mul(k_sin[:], ksw[:], sin_sb[:, s0:s0 + 512])
                    k_cos = rtp.tile([128, 512], bf, tag="tcos")
                    nc.vector.tensor_mul(k_cos[:], kr[:], cos_sb[:, s0:s0 + 512])
                    nc.vector.tensor_add(kt_sb[0:64, 0, 0, s0:s0 + 512],
                                         k_sin[0:64, :], k_cos[0:64, :])
                    nc.vector.tensor_add(kt_sb[64:128, 1, 1, s0:s0 + 512],
                                         k_sin[64:128, :], k_cos[64:128, :])
                    nc.sync.dma_start(kt_sb[64:128, 0, 1, s0:s0 + 512],
                                      kt_sb[0:64, 0, 0, s0:s0 + 512])
                    nc.sync.dma_start(kt_sb[0:64, 1, 0, s0:s0 + 512],
                                      kt_sb[64:128, 1, 1, s0:s0 + 512])

            # ---------------- phase 2: attention + wo ----------------
            with (
                tc.tile_pool(name="pp", bufs=4) as ppool,
                tc.tile_pool(name="rcp", bufs=3) as rcp,
                tc.tile_pool(name="otn", bufs=2) as otp,
                tc.tile_pool(name="wsb", bufs=4) as wsp,
                tc.tile_pool(name="ps_s", bufs=2, space="PSUM") as pss,
                tc.tile_pool(name="ps_o", bufs=1, space="PSUM") as pso,
                tc.tile_pool(name="ps_w", bufs=2, space="PSUM") as psw2,
            ):
                for qs in range(QS_TILES):
                    q0 = qs * 512
                    kts = sched[qs]
                    last_kt = max(e[0] for e in kts)
                    otn_t = otp.tile([128, NPAIR, 512], bf)  # [dh-pair, pair, q]
                    for p in range(NPAIR):
                        kv = p // 2
                        ot0 = pso.tile([128, 512], f32, tag="ot0", name="ot0")
                        ot1 = pso.tile([128, 512], f32, tag="ot1", name="ot1")
                        # software-pipelined by one kt: emit scores(kt) before
                        # PV(kt-1) so the in-order PE queue always has ready
                        # work while the exp of the newest block runs
                        def emit_pv(kt_, jlo_, pt_):
                            for h2, ot in ((0, ot0), (1, ot1)):
                                nc.tensor.matmul(
                                    ot[:, jlo_ * 128:512],
                                    lhsT=vone_sb[:, kv, kt_, :],
                                    rhs=pt_[:, h2, jlo_ * 128:512],
                                    start=(kt_ == kts[0][0]),
                                    stop=(kt_ == last_kt))

                        pend = None
                        for kt, mi, jlo in kts:
                            ql = q0 + jlo * 128
                            spp = pss.tile([128, 2, 512], f32)
                            for h2 in range(2):
                                nc.tensor.matmul(
                                    spp[:, h2, jlo * 128:512],
                                    lhsT=kt_sb[:, kv, h2,
                                               kt * 128:(kt + 1) * 128],
                                    rhs=qt_sb[:, p, ql:q0 + 512],
                                    start=True, stop=True)
                            pt = ppool.tile([128, 2, 512], bf)
                            nc.scalar.activation(pt[:, :, jlo * 128:512],
                                                 spp[:, :, jlo * 128:512], AF.Exp,
                                                 scale=1.0 / np.sqrt(HD))
                            if mi is not None:
                                for h2 in range(2):
                                    nc.vector.tensor_mul(
                                        pt[:, h2, jlo * 128:512],
                                        pt[:, h2, jlo * 128:512],
                                        mt_sb[:, mi, jlo * 128:512])
                            if pend is not None:
                                emit_pv(*pend)
                            pend = (kt, jlo, pt)
                        emit_pv(*pend)
                        # evacuate OT to SBUF with one copy (frees the PSUM
                        # bank for the next pair fast), then normalize off-
                        # PSUM: denom partition 0 -> fast recip -> gpsimd
                        # broadcast -> DVE multiply on the O half (64:128).
                        # Even head writes otn_t[64:128] directly; odd head
                        # goes via SBUF->SBUF DMA to otn_t[0:64].
                        for h2, ot in ((0, ot0), (1, ot1)):
                            otu = rcp.tile([128, 512], f32, tag=f"otu{h2}",
                                           name=f"otu{h2}")
                            nc.vector.tensor_copy(otu[:], ot[:])
                            rc = rcp.tile([128, 512], f32, tag="rc")
                            nc.vector.reciprocal_approx_fast(
                                out=rc[0:1, :], in_=otu[0:1, :])
                            rbs = rcp.tile([128, 512], f32, tag="rbs")
                            nc.gpsimd.partition_broadcast(rbs[:], rc[0:1, :])
                            if h2 == 0:
                                nc.vector.tensor_mul(otn_t[64:128, p, :],
                                                     otu[64:128, :],
                                                     rbs[64:128, :])
                            else:
                                otmp = rcp.tile([128, 512], bf, tag="otmp")
                                nc.vector.tensor_mul(otmp[64:128, :],
                                                     otu[64:128, :],
                                                     rbs[64:128, :])
                                nc.gpsimd.dma_start(otn_t[0:64, p, :],
                                                    otmp[64:128, :])
                    # wo for this q-stripe
                    for j in range(4):
                        sq0 = q0 + j * 128
                        for nb in range(4):
                            wp = psw2.tile([128, 512], f32)
                            for p in range(NPAIR):
                                nc.tensor.matmul(
                                    wp[:],
                                    lhsT=otn_t[:, p, j * 128:(j + 1) * 128],
                                    rhs=wo_sb[:, p, nb * 512:(nb + 1) * 512],
                                    start=(p == 0), stop=(p == NPAIR - 1))
                            wsb = wsp.tile([128, 512], bf)
                            if (j + nb) % 2 == 0:
                                nc.vector.tensor_copy(wsb[:], wp[:])
                            else:
                                nc.scalar.copy(wsb[:], wp[:])
                            nc.sync.dma_start(
                                out.ap()[sq0:sq0 + 128, nb * 512:(nb + 1) * 512],
                                wsb[:])
    nc.compile()
    return nc


def kernel(x, wq, wk, wv, wo, freqs, mask, start_pos):
    sys.path.insert(0, "/opt/trn_rl_repo")
    from concourse.bass_utils import run_bass_kernel_spmd

    x = np.asarray(x, dtype=np.float32)
    per_core, sched, U = _host_prepare(
        x, np.asarray(wq, np.float32), np.asarray(wk, np.float32),
        np.asarray(wv, np.float32), np.asarray(wo, np.float32),
        np.asarray(freqs, np.float32), np.asarray(mask, np.float32))

    nc = _build_program(sched, U)

    trace = bool(int(os.environ.get("BASSKERNEL_TRACE", "0")))
    if trace and "antenv.axon_hooks" not in sys.modules:
        # profile-hook shim (the trimmed antenv package lacks axon_hooks)
        try:
            import types

            if "/root/.axon_site" not in sys.path:
                sys.path.insert(0, "/root/.axon_site")
            from trn_agent_boot.trn_boot import _ntff_profile_via_ctypes

            _hook = _ntff_profile_via_ctypes("/opt/axon/libaxon_pjrt.so")
            _mod = types.ModuleType("antenv.axon_hooks")
            _mod.get_axon_ntff_profile_hook = lambda: _hook
            _mod.set_axon_ntff_profile_hook = lambda h: None
            sys.modules["antenv.axon_hooks"] = _mod
        except Exception:
            trace = False
    res = run_bass_kernel_spmd(nc, per_core, core_ids=list(range(NCORES)),
                               trace=trace)
    if trace:
        kernel._last_exec_time_ns = res.exec_time_ns
        kernel._last_profile = res.profile_json
    acc = np.zeros((B, S, D), np.float64)
    for c in range(NCORES):
        acc[c % 2] += res.results[c]["out"].astype(np.float64)
    return acc.astype(np.float32)


# revision 46
# speedup vs baseline: 1.3548x; 1.0081x over previous
"""GQA prefill attention (B=2, S=2048, D=2048, H=32, KV=8, HD=64) on 8 trn2 cores.

Sharding: batch x head-group. Core c owns batch b=c%2 and head-group hg=c//2:
q-heads [8hg, 8hg+8), kv-heads {2hg, 2hg+1}; computes its partial of
out[b] = attn_out @ wo_hg; host sums the 4 partials per batch.

Device kernel (per core, bf16 matmuls / fp32 PSUM):
  QT[dh,s] per head-pair (two 64-row heads stacked on 128 partitions)
  KT[dh,s] per kv head, duplicated onto both partition halves (row tiling)
  V[k,dh] natural layout via PE transpose; 65th column of ones
  RoPE via pair-swap permutation matmul + elementwise cos/sin tables
  ST[k,q] = KT^T @ QT for a head pair: two concurrent row-tiled K=64 matmuls
  P = exp(ST/8) one ScalarE instr per (pair, kt) over both PSUM banks
  OT[dh|sum, q] += [V | 1]^T-chunks @ P  (N=512 streaming PV, no LDW churn)
  normalize: recip(denom row) -> PE ones-broadcast -> DVE multiply
  out_partial[s,:] = OTn-chunks^T @ wo_hg   (bf16 partial to HBM)
"""

import os
import sys

import numpy as np
import ml_dtypes

BF16 = ml_dtypes.bfloat16

B, S, D, H, KV, HD = 2, 2048, 2048, 32, 8, 64
NCORES = 8
HPC = 8            # q-heads per core
NPAIR = 4          # head pairs per core
QS_TILES = S // 512
KT_TILES = S // 128
DC = D // 128      # contraction chunks for the projections

# per-pair wo row order: odd head first (otn_t partitions 0:64), even second
_WO_PERM = np.concatenate(
    [np.concatenate([np.arange((2 * p + 1) * 64, (2 * p + 2) * 64),
                     np.arange(2 * p * 64, (2 * p + 1) * 64)])
     for p in range(4)])


def _host_prepare(x, wq, wk, wv, wo, freqs, mask):
    """Build per-core device inputs + the mask block schedule."""
    # RoPE tables in the [dh-on-partitions, s] layout used by QT/KT.
    # Two 64-row head copies stacked (head pairs live on 128 partitions).
    # rope: out[2j]   = t[2j] cos - t[2j+1] sin
    #       out[2j+1] = t[2j] sin + t[2j+1] cos
    # with swap(t)[d] = t[d^1]:  out[d] = t[d]*cos[d] + swap(t)[d]*sgn(d)*sin[d]
    c64 = np.cos(freqs.T).repeat(2, axis=0).astype(np.float64)  # [64, S]
    s64 = np.sin(freqs.T).repeat(2, axis=0).astype(np.float64)
    sgn = np.where(np.arange(HD) % 2 == 0, -1.0, 1.0)[:, None]
    cos_t = np.concatenate([c64, c64], axis=0).astype(BF16)           # [128, S]
    sin_t = np.concatenate([s64 * sgn, s64 * sgn], axis=0).astype(BF16)

    # Mask block schedule at [128 k x 512 q] granularity (same for all b, h).
    # Block (qs, kt): full (mask all zero), skip (all <= -30), or masked
    # (multiply exp'd P by exp(mask^T) tile). jlo = first visible 128-q
    # subblock (clean fully-masked prefix only).
    mt_tiles = []  # unique [128, 512] multiplier tiles
    mt_keys = {}
    sched = []  # per qs: list of (kt, mtile_idx | None, jlo)
    for qs in range(QS_TILES):
        lst = []
        for kt in range(KT_TILES):
            blk = mask[qs * 512:(qs + 1) * 512, kt * 128:(kt + 1) * 128]  # [q, k]
            if np.all(blk <= -30.0):
                continue
            jmasked = [np.all(blk[j * 128:(j + 1) * 128] <= -30.0) for j in range(4)]
            jlo = 0
            while jlo < 4 and jmasked[jlo]:
                jlo += 1
            if any(jmasked[jlo:]):
                jlo = 0
            vis = blk[jlo * 128:]
            if np.all(vis == 0.0):
                lst.append((kt, None, jlo))
                continue
            tile_np = np.exp(blk.T.astype(np.float64)).astype(BF16)  # [128k, 512q]
            key = tile_np.tobytes()
            if key not in mt_keys:
                mt_keys[key] = len(mt_tiles)
                mt_tiles.append(tile_np)
            lst.append((kt, mt_keys[key], jlo))
        # every 128-q subblock needs at least one full-width contributing kt
        # so its softmax denominator is well-defined; also the FIRST kt per
        # subblock must be full-width (start=True clears the whole OT bank)
        for j in range(4):
            if not any(e[2] <= j for e in lst):
                lst = [(kt, mi, 0) for (kt, mi, _) in lst]
                break
        if lst and lst[0][2] != 0:
            lst[0] = (lst[0][0], lst[0][1], 0)
        sched.append(lst)
    if not mt_tiles:  # keep the input well-formed even if no masked blocks
        mt_tiles.append(np.ones((128, 512), dtype=BF16))
    mt = np.stack(mt_tiles)  # [U, 128, 512]

    per_core = []
    for c in range(NCORES):
        b, hg = c % 2, c // 2
        per_core.append({
            "xT": np.ascontiguousarray(x[b].T).astype(BF16),            # [D, S]
            "wq": np.ascontiguousarray(
                wq[:, hg * 512:(hg + 1) * 512]).astype(BF16),           # [D, 512]
            "wkv": np.ascontiguousarray(np.concatenate(
                [wk[:, hg * 128:(hg + 1) * 128],
                 wv[:, hg * 128:(hg + 1) * 128]], axis=1)).astype(BF16),  # [D, 256]
            # wo rows permuted to match otn_t layout: pair p holds head 2p+1
            # on partitions 0:64 and head 2p on partitions 64:128
            "wo": np.ascontiguousarray(
                wo[hg * 512:(hg + 1) * 512, :][_WO_PERM, :]).astype(BF16),  # [512, D]
            "cos": cos_t,
            "sin": sin_t,
            "mt": mt,
        })
    return per_core, sched, mt.shape[0]


def _build_program(sched, U):
    import concourse.bass as bass
    import concourse.mybir as mybir
    import concourse.tile as tile
    from concourse import bacc

    dt = mybir.dt
    bf, f32 = dt.bfloat16, dt.float32
    AF = mybir.ActivationFunctionType

    nc = bacc.Bacc("TRN2", target_bir_lowering=False, debug=False,
                   num_devices=NCORES)

    xT = nc.dram_tensor("xT", [D, S], bf, kind="ExternalInput")
    wq = nc.dram_tensor("wq", [D, 512], bf, kind="ExternalInput")
    wkv = nc.dram_tensor("wkv", [D, 256], bf, kind="ExternalInput")
    wo = nc.dram_tensor("wo", [512, D], bf, kind="ExternalInput")
    cos = nc.dram_tensor("cos", [128, S], bf, kind="ExternalInput")
    sin = nc.dram_tensor("sin", [128, S], bf, kind="ExternalInput")
    mt = nc.dram_tensor("mt", [U, 128, 512], bf, kind="ExternalInput")
    out = nc.dram_tensor("out", [S, D], bf, kind="ExternalOutput")

    # pair-swap permutation (block-diag over the two stacked 64-row heads)
    perm_np = np.zeros((128, 128), dtype=BF16)
    for d in range(128):
        perm_np[d ^ 1, d] = 1
    perm_dram = nc.inline_tensor(np.ascontiguousarray(perm_np), name="perm")
    ident_dram = nc.inline_tensor(np.eye(128, dtype=BF16), name="ident")

    with tile.TileContext(nc) as tc:
        with (
            tc.tile_pool(name="const", bufs=1) as cp,
            tc.tile_pool(name="pp", bufs=4) as ppool,
            tc.tile_pool(name="rcp", bufs=2) as rcp,
            tc.tile_pool(name="otn", bufs=2) as otp,
            tc.tile_pool(name="wsb", bufs=4) as wsp,
        ):
            # DMA issue order matters: the first projection matmuls need
            # x chunk 0 + the weight chunks, so those go first; everything
            # needed later (wo, mask tiles, rope tables) queues after.
            wq_sb = cp.tile([128, DC, 512], bf)
            wkv_sb = cp.tile([128, DC, 256], bf)
            xbig0 = cp.tile([128, DC, 512], bf, name="xbig0")
            wq_r = wq.ap().rearrange("(c p) m -> p c m", p=128)
            wkv_r = wkv.ap().rearrange("(c p) m -> p c m", p=128)
            xT_r = xT.ap().rearrange("(c p) s -> p c s", p=128)
            for dc in range(DC):
                nc.sync.dma_start(xbig0[:, dc, :], xT_r[:, dc, 0:512])
                nc.sync.dma_start(wq_sb[:, dc, :], wq_r[:, dc, :])
                nc.sync.dma_start(wkv_sb[:, dc, :], wkv_r[:, dc, :])
            perm_sb = cp.tile([128, 128], bf)
            nc.sync.dma_start(perm_sb[:], perm_dram.ap())
            ident_sb = cp.tile([128, 128], bf)
            nc.sync.dma_start(ident_sb[:], ident_dram.ap())
            cos_sb = cp.tile([128, S], bf)
            nc.sync.dma_start(cos_sb[:], cos.ap())
            sin_sb = cp.tile([128, S], bf)
            nc.sync.dma_start(sin_sb[:], sin.ap())
            mt_sb = cp.tile([128, U, 512], bf)
            nc.sync.dma_start(mt_sb[:], mt.ap().rearrange("u p q -> p u q"))
            wo_sb = cp.tile([128, NPAIR, D], bf)
            nc.sync.dma_start(wo_sb[:], wo.ap().rearrange("(g p) n -> p g n", p=128))
            # warm the ScalarE exp table set now so the first real exp at the
            # phase-1/phase-2 boundary skips the ~2.7us ACT_TABLE_LOAD
            warm_sb = cp.tile([1, 16], f32)
            nc.vector.memset(warm_sb[:], 0.0)
            nc.scalar.activation(warm_sb[0:1, 8:16], warm_sb[0:1, 0:8],
                                 AF.Exp, scale=1.0)

            qt_sb = cp.tile([128, NPAIR, S], bf)    # [dh-pair, pair, s]
            # K^T zero-padded to a full 128 contraction so score matmuls stay
            # in the default 128x128 PE mode (no tiling-mode drains): slot 0
            # = [K | 0] (even head of the pair), slot 1 = [0 | K] (odd head)
            kt_sb = cp.tile([128, 2, 2, S], bf)     # [dh|0 halves, kv, slot, s]
            nc.vector.memset(kt_sb[64:128, :, 0, :], 0.0)
            nc.vector.memset(kt_sb[0:64, :, 1, :], 0.0)
            # PV stationary operand per (kv, kt): [ones(64) | V(64)] so the
            # softmax denominator lands on PSUM partitions 0:64 (partition 0
            # feeds the gpsimd broadcast) and O^T on partitions 64:128
            vone_sb = cp.tile([128, 2, KT_TILES, 128], bf)  # [k%128, kv, kt, 1|dh]
            nc.vector.memset(vone_sb[:, :, :, 0:HD], 1.0)

            # ---------------- phase 1: projections + rope ----------------
            with (
                tc.tile_pool(name="xt", bufs=2) as xp,
                tc.tile_pool(name="raw", bufs=3) as rawp,
                tc.tile_pool(name="rtmp", bufs=3) as rtp,
                tc.tile_pool(name="ps_pr", bufs=1, space="PSUM") as ppr,
                tc.tile_pool(name="ps_sw", bufs=1, space="PSUM") as psw,
                tc.tile_pool(name="ps_vt", bufs=1, space="PSUM") as pvt,
            ):
                for st in range(S // 512):
                    s0 = st * 512
                    if st == 0:
                        xbig = xbig0
                    else:
                        xbig = xp.tile([128, DC, 512], bf)
                        # split the 2MB chunk load across DMA queues
                        for dc2 in range(0, DC, 2):
                            nc.sync.dma_start(xbig[:, dc2:dc2 + 2, :],
                                              xT_r[:, dc2:dc2 + 2, s0:s0 + 512])
                    qps = [ppr.tile([128, 512], f32, tag=f"q{i}", name=f"qp{i}")
                           for i in range(NPAIR)]
                    kp = ppr.tile([128, 512], f32, tag="kp")
                    vp = ppr.tile([128, 512], f32, tag="vp")
                    for dc in range(DC):
                        st_, sp_ = (dc == 0), (dc == DC - 1)
                        for i in range(NPAIR):
                            nc.tensor.matmul(qps[i][:],
                                             lhsT=wq_sb[:, dc, i * 128:(i + 1) * 128],
                                             rhs=xbig[:, dc, :], start=st_, stop=sp_)
                        nc.tensor.matmul(kp[:], lhsT=wkv_sb[:, dc, 0:128],
                                         rhs=xbig[:, dc, :], start=st_, stop=sp_)
                        nc.tensor.matmul(vp[:], lhsT=wkv_sb[:, dc, 128:256],
                                         rhs=xbig[:, dc, :], start=st_, stop=sp_)
                    # raw copies to SBUF (also the swap-matmul inputs),
                    # split across ScalarE and DVE to halve the chain
                    qrs = []
                    for i in range(NPAIR):
                        qr = rawp.tile([128, 512], bf, tag=f"q{i}r", name=f"q{i}r")
                        if i % 2 == 0:
                            nc.scalar.copy(qr[:], qps[i][:])
                        else:
                            nc.vector.tensor_copy(qr[:], qps[i][:])
                        qrs.append(qr)
                    kr = rawp.tile([128, 512], bf, tag="kr")
                    nc.vector.tensor_copy(kr[:], kp[:])
                    vr = rawp.tile([128, 512], bf, tag="vr")
                    nc.scalar.copy(vr[:], vp[:])
                    # V: transpose VT rows back to natural [k, dh]
                    for kv in range(2):
                        pl = kv * 64
                        for j in range(4):
                            vtp = pvt.tile([128, HD], bf)
                            nc.tensor.transpose(
                                vtp[:], vr[pl:pl + 64, j * 128:(j + 1) * 128],
                                ident_sb[pl:pl + 64, pl:pl + 64])
                            nc.vector.tensor_copy(
                                vone_sb[:, kv, 4 * st + j, HD:128], vtp[:])
                    # rope Q (pairs stacked on 128 partitions)
                    for i, qr in enumerate(qrs):
                        swp = psw.tile([128, 512], f32, tag="sw")
                        nc.tensor.matmul(swp[:], lhsT=perm_sb[:], rhs=qr[:],
                                         start=True, stop=True)
                        t_sin = rtp.tile([128, 512], bf, tag="tsin")
                        nc.vector.tensor_mul(t_sin[:], swp[:], sin_sb[:, s0:s0 + 512])
                        t_cos = rtp.tile([128, 512], bf, tag="tcos")
                        nc.vector.tensor_mul(t_cos[:], qr[:], cos_sb[:, s0:s0 + 512])
                        nc.vector.tensor_add(qt_sb[:, i, s0:s0 + 512],
                                             t_sin[:], t_cos[:])
                    # rope K (both kv heads stacked); write into the dup'd
                    # halves, then DMA-duplicate across partition halves
                    ksw = psw.tile([128, 512], f32, tag="sw")
                    nc.tensor.matmul(ksw[:], lhsT=perm_sb[:], rhs=kr[:],
                                     start=True, stop=True)
                    k_sin = rtp.tile([128, 512], bf, tag="tsin")
                    nc.vector.tensor_mul(k_sin[:], ksw[:], sin_sb[:, s0:s0 + 512])
                    k_cos = rtp.tile([128, 512], bf, tag="tcos")
                    nc.vector.tensor_mul(k_cos[:], kr[:], cos_sb[:, s0:s0 + 512])
                    nc.vector.tensor_add(kt_sb[0:64, 0, 0, s0:s0 + 512],
                                         k_sin[0:64, :], k_cos[0:64, :])
                    nc.vector.tensor_add(kt_sb[64:128, 1, 1, s0:s0 + 512],
                                         k_sin[64:128, :], k_cos[64:128, :])
                    nc.sync.dma_start(kt_sb[64:128, 0, 1, s0:s0 + 512],
                                      kt_sb[0:64, 0, 0, s0:s0 + 512])
                    nc.sync.dma_start(kt_sb[0:64, 1, 0, s0:s0 + 512],
                                      kt_sb[64:128, 1, 1, s0:s0 + 512])

            # ---------------- phase 2: attention + wo ----------------
            # SBUF pools for phase 2 were pre-allocated in the outer scope;
            # only the PSUM pools wait on phase-1 bank frees here
            with (
                tc.tile_pool(name="ps_s", bufs=2, space="PSUM") as pss,
                tc.tile_pool(name="ps_o", bufs=1, space="PSUM") as pso,
                tc.tile_pool(name="ps_w", bufs=2, space="PSUM") as psw2,
            ):
                for qs in range(QS_TILES):
                    q0 = qs * 512
                    kts = sched[qs]
                    last_kt = max(e[0] for e in kts)
                    otn_t = otp.tile([128, NPAIR, 512], bf)  # [dh-pair, pair, q]
                    for p in range(NPAIR):
                        kv = p // 2
                        ot0 = pso.tile([128, 512], f32, tag="ot0", name="ot0")
                        ot1 = pso.tile([128, 512], f32, tag="ot1", name="ot1")
                        # software-pipelined by one kt: emit scores(kt) before
                        # PV(kt-1) so the in-order PE queue always has ready
                        # work while the exp of the newest block runs
                        def emit_pv(kt_, jlo_, pt_):
                            for h2, ot in ((0, ot0), (1, ot1)):
                                nc.tensor.matmul(
                                    ot[:, jlo_ * 128:512],
                                    lhsT=vone_sb[:, kv, kt_, :],
                                    rhs=pt_[:, h2, jlo_ * 128:512],
                                    start=(kt_ == kts[0][0]),
                                    stop=(kt_ == last_kt))

                        pend = None
                        for kt, mi, jlo in kts:
                            ql = q0 + jlo * 128
                            spp = pss.tile([128, 2, 512], f32)
                            for h2 in range(2):
                                nc.tensor.matmul(
                                    spp[:, h2, jlo * 128:512],
                                    lhsT=kt_sb[:, kv, h2,
                                               kt * 128:(kt + 1) * 128],
                                    rhs=qt_sb[:, p, ql:q0 + 512],
                                    start=True, stop=True)
                            pt = ppool.tile([128, 2, 512], bf)
                            nc.scalar.activation(pt[:, :, jlo * 128:512],
                                                 spp[:, :, jlo * 128:512], AF.Exp,
                                                 scale=1.0 / np.sqrt(HD))
                            if mi is not None:
                                for h2 in range(2):
                                    nc.vector.tensor_mul(
                                        pt[:, h2, jlo * 128:512],
                                        pt[:, h2, jlo * 128:512],
                                        mt_sb[:, mi, jlo * 128:512])
                            if pend is not None:
                                emit_pv(*pend)
                            pend = (kt, jlo, pt)
                        emit_pv(*pend)
                        # evacuate OT to SBUF with one copy (frees the PSUM
                        # bank for the next pair fast), then normalize off-
                        # PSUM: denom partition 0 -> fast recip -> gpsimd
                        # broadcast -> DVE multiply on the O half (64:128).
                        # Even head writes otn_t[64:128] directly; odd head
                        # goes via SBUF->SBUF DMA to otn_t[0:64].
                        for h2, ot in ((0, ot0), (1, ot1)):
                            otu = rcp.tile([128, 512], f32, tag=f"otu{h2}",
                                           name=f"otu{h2}")
                            nc.vector.tensor_copy(otu[:], ot[:])
                            rc = rcp.tile([128, 512], f32, tag="rc")
                            nc.vector.reciprocal_approx_fast(
                                out=rc[0:1, :], in_=otu[0:1, :])
                            rbs = rcp.tile([128, 512], f32, tag="rbs")
                            nc.gpsimd.partition_broadcast(rbs[:], rc[0:1, :])
                            if h2 == 0:
                                nc.vector.tensor_mul(otn_t[64:128, p, :],
                                                     otu[64:128, :],
                                                     rbs[64:128, :])
                            else:
                                otmp = rcp.tile([128, 512], bf, tag="otmp")
                                nc.vector.tensor_mul(otmp[64:128, :],
                                                     otu[64:128, :],
                                                     rbs[64:128, :])
                                nc.gpsimd.dma_start(otn_t[0:64, p, :],
                                                    otmp[64:128, :])
                    # wo for this q-stripe
                    for j in range(4):
                        sq0 = q0 + j * 128
                        for nb in range(4):
                            wp = psw2.tile([128, 512], f32)
                            for p in range(NPAIR):
                                nc.tensor.matmul(
                                    wp[:],
                                    lhsT=otn_t[:, p, j * 128:(j + 1) * 128],
                                    rhs=wo_sb[:, p, nb * 512:(nb + 1) * 512],
                                    start=(p == 0), stop=(p == NPAIR - 1))
                            wsb = wsp.tile([128, 512], bf)
                            if (j + nb) % 2 == 0:
                                nc.vector.tensor_copy(wsb[:], wp[:])
                            else:
                                nc.scalar.copy(wsb[:], wp[:])
                            nc.sync.dma_start(
                                out.ap()[sq0:sq0 + 128, nb * 512:(nb + 1) * 512],
                                wsb[:])
    nc.compile()
    return nc


def kernel(x, wq, wk, wv, wo, freqs, mask, start_pos):
    sys.path.insert(0, "/opt/trn_rl_repo")
    from concourse.bass_utils import run_bass_kernel_spmd

    x = np.asarray(x, dtype=np.float32)
    per_core, sched, U = _host_prepare(
        x, np.asarray(wq, np.float32), np.asarray(wk, np.float32),
        np.asarray(wv, np.float32), np.asarray(wo, np.float32),
        np.asarray(freqs, np.float32), np.asarray(mask, np.float32))

    nc = _build_program(sched, U)

    trace = bool(int(os.environ.get("BASSKERNEL_TRACE", "0")))
    if trace and "antenv.axon_hooks" not in sys.modules:
        # profile-hook shim (the trimmed antenv package lacks axon_hooks)
        try:
            import types

            if "/root/.axon_site" not in sys.path:
                sys.path.insert(0, "/root/.axon_site")
            from trn_agent_boot.trn_boot import _ntff_profile_via_ctypes

            _hook = _ntff_profile_via_ctypes("/opt/axon/libaxon_pjrt.so")
            _mod = types.ModuleType("antenv.axon_hooks")
            _mod.get_axon_ntff_profile_hook = lambda: _hook
            _mod.set_axon_ntff_profile_hook = lambda h: None
            sys.modules["antenv.axon_hooks"] = _mod
        except Exception:
            trace = False
    res = run_bass_kernel_spmd(nc, per_core, core_ids=list(range(NCORES)),
                               trace=trace)
    if trace:
        kernel._last_exec_time_ns = res.exec_time_ns
        kernel._last_profile = res.profile_json
    acc = np.zeros((B, S, D), np.float64)
    for c in range(NCORES):
        acc[c % 2] += res.results[c]["out"].astype(np.float64)
    return acc.astype(np.float32)
